# revision 15
# baseline (speedup 1.0000x reference)
"""Trainium2 Bass kernel for nn_DotAttention (B=8 data-parallel over 8 cores).

Per core (one batch element), using a jm/jx "permuted block layout":
column c = n*128 + q of any T-layout tile corresponds to row 16q + n of the
natural tensor (from the contiguous "(p n) d" DMA).  The permutation is
consistent across scores / exp / U / gating and undone by the output store
pattern, so it never needs an explicit fixup.

  xp = relu(x @ Wi + bi)          [2048, 96]   (fp16 matmul, DVE relu -> fp8)
  mp = relu(m @ Wm + bm)          [2048, 96]   (fp16 matmul, ACT relu -> fp8)
  S.T[jm, jx] = mp . xp           fp8 DoubleRow matmul (K = 2 x 64)
  E = exp(S.T/sqrt(96) + maskbias)  ACT, fp8 out, two j-tiles per e8 pair
  U.T[d, jx] = mtilde.T @ E       fp8 DoubleRow matmul (2 jm-tiles per inst;
                                   mtilde = [m | pad | 1], so U2 row 32 is the
                                   softmax denominator)
  attn = U / denom                denom reciprocal via PE-transpose to [128,8]
                                   (partition-parallel DVE reciprocal), PE ones
                                   broadcast back to [128, jx]
  out = sigmoid(Wg.T @ res + bg) * res
      = 0.5*(1 + tanh(z/2)) * res   tanh lives in the same ACT table set as
                                   exp, so there are no table reloads; the
                                   (gs+1)*res runs on DVE and the 0.5 folds
                                   into the final PSUM->SBUF copy.

Matmul operands are fp16 except scores/U (fp8e4 DoubleRow); PSUM accumulation
is fp32 everywhere.  The whole tail runs in fp16 (5e-4 relative error budget,
gate threshold is 2e-2).
"""

import math
import os

import numpy as np

import concourse.bass as bass
import concourse.mybir as mybir
import concourse.tile as tile
from concourse import bacc
from concourse.bass_utils import run_bass_kernel_spmd
from concourse.masks import make_identity

F32 = mybir.dt.float32
F16 = mybir.dt.float16
F8 = mybir.dt.float8e4
I32 = mybir.dt.int32
DR = mybir.MatmulPerfMode.DoubleRow

B = 8
JX = 2048
JM = 2048
D = 150
H = 96
G = 300
NB = 16  # 128-column blocks per 2048
HALF = 1024
SCALE = 1.0 / math.sqrt(float(H))
NEG_BIG = 1.0e30


DEBUG_DUMP = bool(os.environ.get("KDBG"))


def _body(tc, x_d, m_d, mask_d, wi_d, bi_d, wm_d, bm_d, wg_d, bg_d, o_d,
          dbg=None):
    nc = tc.nc
    Relu = mybir.ActivationFunctionType.Relu
    Exp = mybir.ActivationFunctionType.Exp
    Tanh = mybir.ActivationFunctionType.Tanh
    Copy = mybir.ActivationFunctionType.Copy
    Add = mybir.AluOpType.add
    Max = mybir.AluOpType.max
    Mult = mybir.AluOpType.mult
    Sub = mybir.AluOpType.subtract

    import contextlib

    with contextlib.ExitStack() as ctx:
        const = ctx.enter_context(tc.tile_pool(name="const", bufs=1))
        work = ctx.enter_context(tc.tile_pool(name="work", bufs=2))
        epool = ctx.enter_context(tc.tile_pool(name="epool", bufs=3))
        psb = ctx.enter_context(tc.tile_pool(name="psb", bufs=2, space="PSUM"))
        psu = ctx.enter_context(tc.tile_pool(name="psu", bufs=1, space="PSUM"))

        # ---- input DMAs first: they are the longest poles ----------------
        # x/m in "(p n) d" layout: 9600B contiguous per partition.
        x_nat = const.tile([128, NB, D], F32)
        m_nat = const.tile([128, NB, D], F32)
        x_re = x_d.rearrange("(p n) d -> p n d", n=NB)
        m_re = m_d.rearrange("(p n) d -> p n d", n=NB)
        for c in range(4):
            cs = slice(c * 4, (c + 1) * 4)
            nc.scalar.dma_start(out=m_nat[:, cs, :], in_=m_re[:, cs, :])
            nc.sync.dma_start(out=x_nat[:, cs, :], in_=x_re[:, cs, :])
        mask_sb = const.tile([128, NB], I32)
        nc.gpsimd.dma_start(
            out=mask_sb, in_=mask_d.rearrange("(p n) -> p n", n=NB)
        )
        wstage = const.tile([128, 2 * H], F32)
        nc.gpsimd.dma_start(out=wstage[:, 0:H], in_=wi_d[0:128, :])
        nc.gpsimd.dma_start(out=wstage[:, H : 2 * H], in_=wm_d[0:128, :])
        wstage2 = const.tile([D - 128, 2 * H], F32)
        nc.gpsimd.dma_start(out=wstage2[:, 0:H], in_=wi_d[128:D, :])
        nc.gpsimd.dma_start(out=wstage2[:, H : 2 * H], in_=wm_d[128:D, :])
        bstage = const.tile([H, 2], F32)
        nc.gpsimd.dma_start(
            out=bstage[:, 0:1], in_=bi_d.rearrange("(n one) -> n one", one=1)
        )
        nc.gpsimd.dma_start(
            out=bstage[:, 1:2], in_=bm_d.rearrange("(n one) -> n one", one=1)
        )
        # Wg / bg are needed only ~40us in; load on the vector queue.
        wgst = []
        for gi, (g0, g1) in enumerate([(0, 128), (128, D), (D, D + 128), (D + 128, G)]):
            wst = const.tile([g1 - g0, G], F32, tag=f"wgst_{gi}", name=f"wgst_{gi}")
            nc.gpsimd.dma_start(out=wst, in_=wg_d[g0:g1, :])
            wgst.append(wst)
        bg_st = []
        for gi, (g0, g1) in enumerate([(0, 128), (128, 256), (256, G)]):
            t = const.tile([g1 - g0, 1], F32, tag=f"bgst_{gi}", name=f"bgst_{gi}")
            nc.gpsimd.dma_start(
                out=t, in_=bg_d[g0:g1].rearrange("(n one) -> n one", one=1)
            )
            bg_st.append(t)

        # ---- constants ---------------------------------------------------
        ident16 = const.tile([128, 128], F16)
        make_identity(nc, ident16)
        ident32 = const.tile([128, 128], F32)
        make_identity(nc, ident32)
        ones16 = const.tile([1, 128], F16)
        nc.gpsimd.memset(ones16, 1.0)
        # preload the exp ACT table while DMAs land (tanh/relu/copy share it)
        dummy = const.tile([1, 1], F32)
        nc.scalar.activation(
            out=dummy, in_=ident32[0:1, 0:1], func=Exp, scale=1.0
        )

        # ---- weight prep -------------------------------------------------
        wi16 = const.tile([128, 128], F16)  # cols 96:128 zero-padded
        wm16 = const.tile([128, 128], F16)
        nc.gpsimd.memset(wi16[:, H:128], 0.0)
        nc.gpsimd.memset(wm16[:, H:128], 0.0)
        nc.vector.tensor_copy(out=wi16[:, 0:H], in_=wstage[:, 0:H])
        nc.vector.tensor_copy(out=wm16[:, 0:H], in_=wstage[:, H : 2 * H])
        wi16b = const.tile([D - 128, 128], F16)
        wm16b = const.tile([D - 128, 128], F16)
        nc.gpsimd.memset(wi16b[:, H:128], 0.0)
        nc.gpsimd.memset(wm16b[:, H:128], 0.0)
        nc.vector.tensor_copy(out=wi16b[:, 0:H], in_=wstage2[:, 0:H])
        nc.vector.tensor_copy(out=wm16b[:, 0:H], in_=wstage2[:, H : 2 * H])
        bi128 = const.tile([128, 2], F32)  # col 0 = bi, col 1 = bm; rows 96+ = 0
        nc.gpsimd.memset(bi128[H:128, :], 0.0)
        nc.gpsimd.tensor_copy(out=bi128[0:H, :], in_=bstage)

        # mask -> additive exp bias (mask-1)*1e30, natural layout (no transpose)
        maskf = const.tile([128, NB], F32)
        nc.vector.tensor_copy(out=maskf, in_=mask_sb)
        nc.vector.tensor_scalar(
            out=maskf, in0=maskf, scalar1=1.0, scalar2=NEG_BIG,
            op0=Sub, op1=Mult,
        )

        # mtilde fp8 U stationaries, (j, d)-contiguous so the DoubleRow
        # weight slices [:, 2jp:2jp+2, :] merge into one run (ISA req):
        # m8a = m cols 0:128; m8b = [m cols 128:150 | 0-pad | 1] (denominator
        # column lands at U2 partition 32).
        m8a = const.tile([128, NB, 128], F8)
        m8b = const.tile([128, NB, 64], F8)
        nc.gpsimd.memset(m8b[:, :, 22:32], 0.0)
        nc.gpsimd.memset(m8b[:, :, 32:33], 1.0)
        nc.gpsimd.memset(m8b[:, :, 33:64], 0.0)
        for c in range(2):
            cs = slice(c * 8, (c + 1) * 8)
            nc.gpsimd.tensor_copy(out=m8a[:, cs, :], in_=m_nat[:, cs, 0:128])
            nc.gpsimd.tensor_copy(out=m8b[:, cs, 0:22], in_=m_nat[:, cs, 128:D])

        # ---- fp16 casts + transposes into T layout -----------------------
        m16 = const.tile([128, NB, D], F16)
        x16 = const.tile([128, NB, D], F16)
        mT16 = const.tile([128, JM], F16)
        mT16b = const.tile([D - 128, JM], F16)
        xT16 = const.tile([128, JX], F16)
        xT16b = const.tile([D - 128, JX], F16)
        # projected activations, fp8, DoubleRow layout [64, blk, 2, cols]
        # (k-slot pair dim adjacent to cols so DR slices are mergeable)
        mp8 = const.tile([64, NB, 2, 128], F8)
        xp8 = const.tile([64, 4, 2, 512], F8)

        def tp_group(src16, blk4, dT, dTb, eng):
            # 4 block-transposes batched into one PSUM tile + one copy
            sl4 = slice(blk4 * 128, (blk4 + 4) * 128)
            t1 = psb.tile([128, 512], F16, tag="big", name="tp1")
            t2 = psb.tile([D - 128, 512], F16, tag="big", name="tp2")
            for i in range(4):
                ps = slice(i * 128, (i + 1) * 128)
                nc.tensor.transpose(t1[:, ps], src16[:, blk4 + i, 0:128], ident16)
                nc.tensor.transpose(t2[:, ps], src16[:, blk4 + i, 128:D], ident16)
            (eng.tensor_copy if eng is not nc.scalar else eng.copy)(
                out=dT[:, sl4], in_=t1
            )
            (eng.tensor_copy if eng is not nc.scalar else eng.copy)(
                out=dTb[:, sl4], in_=t2
            )

        def proj(wa, wb, bcol, srcT, srcTb, dst8, q, relu_eng):
            qs = slice(q * 512, (q + 1) * 512)
            pp = psb.tile([128, 512], F32, tag="big", name="pp")
            nc.tensor.matmul(
                pp, wa, srcT[:, qs], start=True, stop=False, skip_group_check=True
            )
            nc.tensor.matmul(
                pp, wb, srcTb[:, qs], start=False, stop=True, skip_group_check=True
            )
            for i in range(2):
                hs = slice(i * 64, (i + 1) * 64)
                if dst8 is mp8:
                    dview = dst8[:, 4 * q : 4 * q + 4, i, :]
                else:
                    dview = dst8[:, q, i, :]
                if relu_eng is nc.scalar:
                    nc.scalar.activation(
                        out=dview, in_=pp[hs, :], func=Relu,
                        bias=bi128[hs, bcol : bcol + 1], scale=1.0,
                    )
                else:
                    relu_eng.tensor_scalar(
                        out=dview, in0=pp[hs, :],
                        scalar1=bi128[hs, bcol : bcol + 1], scalar2=0.0,
                        op0=Add, op1=Max,
                    )

        # m pipeline: cast chunk -> transposes -> projection quarter
        for c4 in range(0, NB, 4):
            nc.vector.tensor_copy(
                out=m16[:, c4 : c4 + 4, :], in_=m_nat[:, c4 : c4 + 4, :]
            )
            tp_group(m16, c4, mT16, mT16b, nc.vector if c4 % 8 == 0 else nc.scalar)
            proj(wm16, wm16b, 1, mT16, mT16b, mp8, c4 // 4, nc.scalar)
        # x: cast + transpose all blocks, project only quarter 0 up front;
        # quarters 1-3 are interleaved into attention h0's PE slack.
        for c4 in range(0, NB, 4):
            nc.vector.tensor_copy(
                out=x16[:, c4 : c4 + 4, :], in_=x_nat[:, c4 : c4 + 4, :]
            )
            tp_group(x16, c4, xT16, xT16b, nc.scalar if c4 % 8 == 0 else nc.vector)
        proj(wi16, wi16b, 0, xT16, xT16b, xp8, 0, nc.vector)
        proj(wi16, wi16b, 0, xT16, xT16b, xp8, 1, nc.vector)

        # gate weights fp16 (4 g-chunks)
        wg16 = []
        for gi in range(4):
            w = const.tile(
                [wgst[gi].shape[0], G], F16, tag=f"wg16_{gi}", name=f"wg16_{gi}"
            )
            (nc.gpsimd if gi % 2 == 0 else nc.vector).tensor_copy(
                out=w, in_=wgst[gi]
            )
            wg16.append(w)
        bg_half = []
        for gi in range(3):
            t = const.tile(
                [bg_st[gi].shape[0], 1], F32, tag=f"bgh_{gi}", name=f"bgh_{gi}"
            )
            nc.gpsimd.tensor_scalar_mul(out=t, in0=bg_st[gi], scalar1=0.5)
            bg_half.append(t)

        # ---- attention h0 / h1 -------------------------------------------
        # Per half: 16 j-tiles of scores+exp, 8 DoubleRow U pairs.  The exp
        # (ACT) is the rate limiter, so leftover x projections are slotted
        # into h0's PE stream.  U tiles are copied to SBUF right after each
        # half so the single-buffer PSUM pools can be reused.
        U1c, U2c, rr16n = [], [], {}

        def emit_scores(h, j):
            sp = psb.tile([128, HALF], F32, tag="big", name="sp")
            for s in range(2):
                nc.tensor.matmul(
                    sp[:, s * 512 : (s + 1) * 512],
                    mp8[:, j], xp8[:, 2 * h + s],
                    start=True, stop=True, perf_mode=DR, skip_group_check=True,
                )
            return sp

        def emit_exp(sp, j, e8, slot):
            nc.scalar.activation(
                out=e8[:, :, slot, :], in_=sp, func=Exp,
                bias=maskf[:, j : j + 1], scale=SCALE,
            )

        def emit_U(U1, U2, e8, jp):
            first, last = jp == 0, jp == 7
            for s in range(2):
                ps = slice(s * 512, (s + 1) * 512)
                nc.tensor.matmul(
                    U1[:, ps], m8a[:, 2 * jp : 2 * jp + 2], e8[:, s],
                    start=first, stop=last, perf_mode=DR, skip_group_check=True,
                )
                nc.tensor.matmul(
                    U2[:, ps], m8b[:, 2 * jp : 2 * jp + 2], e8[:, s],
                    start=first, stop=last, perf_mode=DR, skip_group_check=True,
                )

        def emit_denomT(h):
            # denom row [1,1024] -> [128, 8] via PE transposes (so the DVE
            # reciprocal runs partition-parallel instead of 6.5ns/elem on a
            # single lane), then PE-transpose back to a [1, 1024] row.
            # The [1,1] identity sits at partition 32 to match the denom
            # row's base partition (engine APs need 32-aligned bases).
            dps = psb.tile([128, 8], F32, tag="big", name="dps")
            for c in range(8):
                nc.tensor.transpose(
                    dps[:, c : c + 1],
                    U2c[h][32:33, c * 128 : (c + 1) * 128],
                    ident32[32:33, 32:33],
                )
            rrT = work.tile([128, 8], F32, tag="rrT")
            nc.vector.reciprocal(out=rrT, in_=dps)
            rrT16 = work.tile([128, 8], F16, tag="rrT16")
            nc.vector.tensor_copy(out=rrT16, in_=rrT)
            rrps = psb.tile([1, HALF], F16, tag="big", name="rrps")
            for c in range(8):
                nc.tensor.transpose(
                    rrps[:, c * 128 : (c + 1) * 128],
                    rrT16[:, c : c + 1],
                    ident16,
                )
            rr = work.tile([1, HALF], F16, tag="rr16")
            nc.vector.tensor_copy(out=rr[:, 0:512], in_=rrps[:, 0:512])
            nc.scalar.copy(out=rr[:, 512:HALF], in_=rrps[:, 512:HALF])
            rr16n[h] = rr

        for h in range(2):
            U1 = psu.tile([128, HALF], F32, tag="u1")
            U2 = psu.tile([64, HALF], F32, tag="u2")
            sps, e8s = {}, {}
            e8s[0] = epool.tile([128, 2, 2, 512], F8, tag="e8", name="e8")
            sps[0] = emit_scores(h, 0)
            if dbg and h == 0:
                sp0dbg = const.tile([128, HALF], F32)
                nc.vector.tensor_copy(out=sp0dbg, in_=sps[0])
            sps[1] = emit_scores(h, 1)
            for jp in range(8):
                emit_exp(sps.pop(2 * jp), 2 * jp, e8s[jp], 0)
                if 2 * jp + 2 < 16:
                    sps[2 * jp + 2] = emit_scores(h, 2 * jp + 2)
                emit_exp(sps.pop(2 * jp + 1), 2 * jp + 1, e8s[jp], 1)
                if 2 * jp + 3 < 16:
                    sps[2 * jp + 3] = emit_scores(h, 2 * jp + 3)
                if jp + 1 < 8:
                    e8s[jp + 1] = epool.tile([128, 2, 2, 512], F8, tag="e8", name="e8")
                e8cur = e8s.pop(jp)
                if dbg and h == 0 and jp == 0:
                    e8dbg = const.tile([128, 2, 2, 512], F32)
                    nc.vector.tensor_copy(out=e8dbg, in_=e8cur)
                emit_U(U1, U2, e8cur, jp)
                # interleaved leftover work (PE has slack vs ACT):
                if h == 0 and jp in (1, 3):
                    proj(
                        wi16, wi16b, 0, xT16, xT16b, xp8,
                        2 + (jp - 1) // 2, nc.vector,
                    )
                if h == 1 and jp == 2:
                    emit_denomT(0)
            u1c = work.tile([128, HALF], F32, tag="u1c")
            nc.vector.tensor_copy(out=u1c, in_=U1)
            u2c = work.tile([33, HALF], F32, tag="u2c")
            nc.scalar.copy(out=u2c, in_=U2[0:33, :])
            U1c.append(u1c)
            U2c.append(u2c)

        # ---- gating + store tails ----------------------------------------
        o_re = o_d.rearrange("(q n) k -> q n k", n=NB)
        kranges = [(0, 128), (128, 256), (256, G)]
        for h in range(2):
            hs = slice(h * HALF, (h + 1) * HALF)
            # broadcast 1/denom to all partitions via PE ones-matmul
            bc = psb.tile([128, HALF], F32, tag="big", name="bc")
            for s in range(2):
                ps = slice(s * 512, (s + 1) * 512)
                nc.tensor.matmul(
                    bc[:, ps], ones16, rr16n[h][:, ps],
                    start=True, stop=True, skip_group_check=True,
                )
            RC16 = work.tile([128, HALF], F16, tag="RC16")
            nc.vector.tensor_mul(out=RC16, in0=U1c[h], in1=bc)
            RD16 = work.tile([D - 128, HALF], F16, tag="RD16")
            nc.vector.tensor_mul(
                out=RD16, in0=U2c[h][0 : D - 128, :], in1=bc[0 : D - 128, :]
            )
            # output-aligned res staging (partition shifts via DMA)
            res_mid = work.tile([128, HALF], F16, tag="res_mid")
            nc.vector.tensor_copy(out=res_mid[0 : D - 128, :], in_=xT16b[:, hs])
            nc.sync.dma_start(
                out=res_mid[D - 128 : 128, :], in_=RC16[0 : 256 - D, :]
            )
            res_hi = work.tile([G - 256, HALF], F16, tag="res_hi")
            nc.scalar.dma_start(
                out=res_hi[0 : D - 128, :], in_=RC16[256 - D : 128, :]
            )
            nc.sync.dma_start(out=res_hi[D - 128 : G - 256, :], in_=RD16)
            res_g = [xT16[:, hs], xT16b[:, hs], RC16, RD16]
            res_k = [xT16[:, hs], res_mid, res_hi]
            oT = [
                work.tile([128, HALF], F16, tag="oT0", name="oT0"),
                work.tile([128, HALF], F16, tag="oT1", name="oT1"),
                work.tile([G - 256, HALF], F16, tag="oT2", name="oT2"),
            ]
            for kc, (k0, k1) in enumerate(kranges):
                kw = k1 - k0
                gp = psb.tile([kw, HALF], F32, tag="big", name="gp")
                for s in range(2):
                    ps = slice(s * 512, (s + 1) * 512)
                    for gc in range(4):
                        nc.tensor.matmul(
                            gp[:, ps], wg16[gc][:, k0:k1], res_g[gc][:, ps],
                            start=(gc == 0), stop=(gc == 3),
                            skip_group_check=True,
                        )
                if h == 0 and kc == 0:
                    emit_denomT(1)
                # gate = sigmoid(z) = 0.5*(1+tanh(z/2)); tanh shares the exp
                # ACT table.  (gs+1) folds into the DVE multiply, 0.5 into
                # the final copy.
                gs = work.tile([kw, HALF], F16, tag="gs", bufs=3)
                nc.scalar.activation(
                    out=gs, in_=gp, func=Tanh, bias=bg_half[kc], scale=0.5
                )
                nc.vector.scalar_tensor_tensor(
                    out=oT[kc], in0=gs, scalar=1.0, in1=res_k[kc],
                    op0=Add, op1=Mult,
                )
            # transpose back (fp16), scale by 0.5 on the upconverting copy,
            # store 4 blocks per DMA
            for g4 in range(2):
                onat = work.tile([128, 4, 304], F32, tag="onat", bufs=2)
                for i in range(2):
                    op2 = psb.tile([128, 2, 304], F16, tag="big", name="op2")
                    for k in range(2):
                        n_loc = g4 * 4 + i * 2 + k
                        sl = slice(n_loc * 128, (n_loc + 1) * 128)
                        nc.tensor.transpose(
                            op2[:, k, 0:128], oT[0][:, sl], ident16
                        )
                        nc.tensor.transpose(
                            op2[:, k, 128:256], oT[1][:, sl], ident16
                        )
                        nc.tensor.transpose(
                            op2[:, k, 256:G], oT[2][:, sl],
                            ident16[: G - 256, : G - 256],
                        )
                    osl = slice(i * 2, (i + 1) * 2)
                    if i == 0:
                        nc.vector.tensor_scalar_mul(
                            out=onat[:, osl, :], in0=op2, scalar1=0.5
                        )
                    else:
                        nc.scalar.activation(
                            out=onat[:, osl, :], in_=op2, func=Copy,
                            bias=0.0, scale=0.5,
                        )
                n0 = h * 8 + g4 * 4
                dq = nc.sync if g4 % 2 == 0 else nc.scalar
                dq.dma_start(
                    out=o_re[:, n0 : n0 + 4, :], in_=onat[:, :, 0:G]
                )

        if dbg:
            mpf = const.tile([64, NB, 2, 128], F32)
            nc.vector.tensor_copy(out=mpf, in_=mp8)
            nc.sync.dma_start(out=dbg["mp"][:], in_=mpf)
            xpf = const.tile([64, 4, 2, 512], F32)
            nc.vector.tensor_copy(out=xpf, in_=xp8)
            nc.sync.dma_start(out=dbg["xp"][:], in_=xpf)
            nc.sync.dma_start(out=dbg["u1"][:], in_=U1c[0])
            nc.sync.dma_start(out=dbg["u2"][:], in_=U2c[0])
            rrf = const.tile([1, HALF], F32)
            nc.vector.tensor_copy(out=rrf, in_=rr16n[0])
            nc.sync.dma_start(out=dbg["rr"][:], in_=rrf)
            nc.sync.dma_start(out=dbg["sp0"][:], in_=sp0dbg)
            nc.sync.dma_start(out=dbg["e8"][:], in_=e8dbg)
            m8af = const.tile([128, NB, 128], F32)
            nc.vector.tensor_copy(out=m8af, in_=m8a)
            nc.sync.dma_start(out=dbg["m8a"][:], in_=m8af)
            nc.sync.dma_start(out=dbg["maskf"][:], in_=maskf)


_NC_CACHE = None


def _build_nc():
    global _NC_CACHE
    if _NC_CACHE is not None:
        return _NC_CACHE
    nc = bacc.Bacc(None, target_bir_lowering=False, debug=False)
    x_d = nc.dram_tensor("x", [JX, D], F32, kind="ExternalInput")
    m_d = nc.dram_tensor("m", [JM, D], F32, kind="ExternalInput")
    mask_d = nc.dram_tensor("mask", [JM], I32, kind="ExternalInput")
    wi_d = nc.dram_tensor("Wi", [D, H], F32, kind="ExternalInput")
    bi_d = nc.dram_tensor("bi", [H], F32, kind="ExternalInput")
    wm_d = nc.dram_tensor("Wm", [D, H], F32, kind="ExternalInput")
    bm_d = nc.dram_tensor("bm", [H], F32, kind="ExternalInput")
    wg_d = nc.dram_tensor("Wg", [G, G], F32, kind="ExternalInput")
    bg_d = nc.dram_tensor("bg", [G], F32, kind="ExternalInput")
    o_d = nc.dram_tensor("out", [JX, G], F32, kind="ExternalOutput")
    dbg = None
    if DEBUG_DUMP:
        dbg = {
            "mp": nc.dram_tensor("dbg_mp", [64, NB, 2, 128], F32, kind="ExternalOutput"),
            "xp": nc.dram_tensor("dbg_xp", [64, 4, 2, 512], F32, kind="ExternalOutput"),
            "u1": nc.dram_tensor("dbg_u1", [128, HALF], F32, kind="ExternalOutput"),
            "u2": nc.dram_tensor("dbg_u2", [33, HALF], F32, kind="ExternalOutput"),
            "rr": nc.dram_tensor("dbg_rr", [1, HALF], F32, kind="ExternalOutput"),
            "sp0": nc.dram_tensor("dbg_sp0", [128, HALF], F32, kind="ExternalOutput"),
            "e8": nc.dram_tensor("dbg_e8", [128, 2, 2, 512], F32, kind="ExternalOutput"),
            "m8a": nc.dram_tensor("dbg_m8a", [128, NB, 128], F32, kind="ExternalOutput"),
            "maskf": nc.dram_tensor("dbg_maskf", [128, NB], F32, kind="ExternalOutput"),
        }
    with tile.TileContext(nc) as tc:
        _body(tc, x_d, m_d, mask_d, wi_d, bi_d, wm_d, bm_d, wg_d, bg_d, o_d,
              dbg=dbg)
    nc.finalize()
    _NC_CACHE = nc
    return nc


def _in_maps(inputs, memory, mask, Wi, bi, Wm, bm, Wg, bg):
    maps = []
    for b in range(B):
        maps.append(
            {
                "x": np.ascontiguousarray(inputs[b], dtype=np.float32),
                "m": np.ascontiguousarray(memory[b], dtype=np.float32),
                "mask": np.ascontiguousarray(mask[b], dtype=np.int32),
                "Wi": np.ascontiguousarray(Wi, dtype=np.float32),
                "bi": np.ascontiguousarray(bi, dtype=np.float32),
                "Wm": np.ascontiguousarray(Wm, dtype=np.float32),
                "bm": np.ascontiguousarray(bm, dtype=np.float32),
                "Wg": np.ascontiguousarray(Wg, dtype=np.float32),
                "bg": np.ascontiguousarray(bg, dtype=np.float32),
            }
        )
    return maps


def run_spmd(inputs, memory, mask, Wi, bi, Wm, bm, Wg, bg, **spmd_kwargs):
    """Run the kernel across 8 cores; returns the BassKernelResults."""
    nc = _build_nc()
    maps = _in_maps(
        np.asarray(inputs), np.asarray(memory), np.asarray(mask),
        np.asarray(Wi), np.asarray(bi), np.asarray(Wm), np.asarray(bm),
        np.asarray(Wg), np.asarray(bg),
    )
    return run_bass_kernel_spmd(nc, maps, list(range(B)), **spmd_kwargs)


def kernel(inputs, memory, mask, Wi, bi, Wm, bm, Wg, bg):
    res = run_spmd(inputs, memory, mask, Wi, bi, Wm, bm, Wg, bg)
    out = np.stack([res.results[b]["out"] for b in range(B)], axis=0)
    return out.astype(np.float32)


# revision 17
# speedup vs baseline: 1.0193x; 1.0193x over previous
"""Trainium2 Bass kernel for nn_DotAttention (B=8 data-parallel over 8 cores).

Per core (one batch element), using a jm/jx "permuted block layout":
column c = n*128 + q of any T-layout tile corresponds to row 16q + n of the
natural tensor (from the contiguous "(p n) d" DMA).  The permutation is
consistent across scores / exp / U / gating and undone by the output store
pattern, so it never needs an explicit fixup.

  xp = relu(x @ Wi + bi)          [2048, 96]   (fp16 matmul, DVE relu -> fp8)
  mp = relu(m @ Wm + bm)          [2048, 96]   (fp16 matmul, ACT relu -> fp8)
  S.T[jm, jx] = mp . xp           fp8 DoubleRow matmul (K = 2 x 64)
  E = exp(S.T/sqrt(96) + maskbias)  ACT, fp8 out, two j-tiles per e8 pair
  U.T[d, jx] = mtilde.T @ E       fp8 DoubleRow matmul (2 jm-tiles per inst;
                                   mtilde = [m | pad | 1], so U2 row 32 is the
                                   softmax denominator)
  attn = U / denom                denom reciprocal via PE-transpose to [128,8]
                                   (partition-parallel DVE reciprocal), PE ones
                                   broadcast back to [128, jx]
  out = sigmoid(Wg.T @ res + bg) * res
      = 0.5*(1 + tanh(z/2)) * res   tanh lives in the same ACT table set as
                                   exp, so there are no table reloads; the
                                   (gs+1)*res runs on DVE and the 0.5 folds
                                   into the final PSUM->SBUF copy.

Matmul operands are fp16 except scores/U (fp8e4 DoubleRow); PSUM accumulation
is fp32 everywhere.  The whole tail runs in fp16 (5e-4 relative error budget,
gate threshold is 2e-2).
"""

import math
import os

import numpy as np

import concourse.bass as bass
import concourse.mybir as mybir
import concourse.tile as tile
from concourse import bacc
from concourse.bass_utils import run_bass_kernel_spmd
from concourse.masks import make_identity

F32 = mybir.dt.float32
F16 = mybir.dt.float16
F8 = mybir.dt.float8e4
I32 = mybir.dt.int32
DR = mybir.MatmulPerfMode.DoubleRow

B = 8
JX = 2048
JM = 2048
D = 150
H = 96
G = 300
NB = 16  # 128-column blocks per 2048
HALF = 1024
SCALE = 1.0 / math.sqrt(float(H))
NEG_BIG = 1.0e30


DEBUG_DUMP = bool(os.environ.get("KDBG"))


def _body(tc, x_d, m_d, mask_d, wi_d, bi_d, wm_d, bm_d, wg_d, bg_d, o_d,
          dbg=None):
    nc = tc.nc
    Relu = mybir.ActivationFunctionType.Relu
    Exp = mybir.ActivationFunctionType.Exp
    Tanh = mybir.ActivationFunctionType.Tanh
    Copy = mybir.ActivationFunctionType.Copy
    Add = mybir.AluOpType.add
    Max = mybir.AluOpType.max
    Mult = mybir.AluOpType.mult
    Sub = mybir.AluOpType.subtract

    import contextlib

    with contextlib.ExitStack() as ctx:
        const = ctx.enter_context(tc.tile_pool(name="const", bufs=1))
        work = ctx.enter_context(tc.tile_pool(name="work", bufs=2))
        epool = ctx.enter_context(tc.tile_pool(name="epool", bufs=3))
        psb = ctx.enter_context(tc.tile_pool(name="psb", bufs=2, space="PSUM"))
        psu = ctx.enter_context(tc.tile_pool(name="psu", bufs=1, space="PSUM"))

        # ---- input DMAs first: they are the longest poles ----------------
        # x/m in "(p n) d" layout: 9600B contiguous per partition.
        x_nat = const.tile([128, NB, D], F32)
        m_nat = const.tile([128, NB, D], F32)
        x_re = x_d.rearrange("(p n) d -> p n d", n=NB)
        m_re = m_d.rearrange("(p n) d -> p n d", n=NB)
        for c in range(4):
            cs = slice(c * 4, (c + 1) * 4)
            nc.scalar.dma_start(out=m_nat[:, cs, :], in_=m_re[:, cs, :])
            nc.sync.dma_start(out=x_nat[:, cs, :], in_=x_re[:, cs, :])
        mask_sb = const.tile([128, NB], I32)
        wstage = const.tile([128, 2 * H], F32)
        wstage2 = const.tile([D - 128, 2 * H], F32)
        bstage = const.tile([H, 2], F32)

        # ---- constants (emitted before the gpsimd DMA dispatches so the
        # identity build isn't stuck behind ~10 serialized 0.7us dispatches)
        ident16 = const.tile([128, 128], F16)
        make_identity(nc, ident16)
        ident32 = const.tile([128, 128], F32)
        make_identity(nc, ident32)
        ones16 = const.tile([1, 128], F16)
        nc.gpsimd.memset(ones16, 1.0)
        # preload the exp ACT table while DMAs land (tanh/relu/copy share it)
        dummy = const.tile([1, 1], F32)
        nc.scalar.activation(
            out=dummy, in_=ident32[0:1, 0:1], func=Exp, scale=1.0
        )

        # small-weight DMAs on the gpsimd queue (inputs own sync/scalar)
        nc.gpsimd.dma_start(out=wstage[:, 0:H], in_=wi_d[0:128, :])
        nc.gpsimd.dma_start(out=wstage[:, H : 2 * H], in_=wm_d[0:128, :])
        nc.gpsimd.dma_start(out=wstage2[:, 0:H], in_=wi_d[128:D, :])
        nc.gpsimd.dma_start(out=wstage2[:, H : 2 * H], in_=wm_d[128:D, :])
        nc.gpsimd.dma_start(
            out=bstage[:, 0:1], in_=bi_d.rearrange("(n one) -> n one", one=1)
        )
        nc.gpsimd.dma_start(
            out=bstage[:, 1:2], in_=bm_d.rearrange("(n one) -> n one", one=1)
        )
        nc.gpsimd.dma_start(
            out=mask_sb, in_=mask_d.rearrange("(p n) -> p n", n=NB)
        )
        # Wg / bg are needed only ~40us in; queue them behind the inputs
        wgst = []
        for gi, (g0, g1) in enumerate([(0, 128), (128, D), (D, D + 128), (D + 128, G)]):
            wst = const.tile([g1 - g0, G], F32, tag=f"wgst_{gi}", name=f"wgst_{gi}")
            (nc.sync if gi % 2 == 0 else nc.scalar).dma_start(
                out=wst, in_=wg_d[g0:g1, :]
            )
            wgst.append(wst)
        bg_st = []
        for gi, (g0, g1) in enumerate([(0, 128), (128, 256), (256, G)]):
            t = const.tile([g1 - g0, 1], F32, tag=f"bgst_{gi}", name=f"bgst_{gi}")
            nc.scalar.dma_start(
                out=t, in_=bg_d[g0:g1].rearrange("(n one) -> n one", one=1)
            )
            bg_st.append(t)

        # ---- weight prep -------------------------------------------------
        wi16 = const.tile([128, 128], F16)  # cols 96:128 zero-padded
        wm16 = const.tile([128, 128], F16)
        nc.gpsimd.memset(wi16[:, H:128], 0.0)
        nc.gpsimd.memset(wm16[:, H:128], 0.0)
        nc.vector.tensor_copy(out=wi16[:, 0:H], in_=wstage[:, 0:H])
        nc.vector.tensor_copy(out=wm16[:, 0:H], in_=wstage[:, H : 2 * H])
        wi16b = const.tile([D - 128, 128], F16)
        wm16b = const.tile([D - 128, 128], F16)
        nc.gpsimd.memset(wi16b[:, H:128], 0.0)
        nc.gpsimd.memset(wm16b[:, H:128], 0.0)
        nc.vector.tensor_copy(out=wi16b[:, 0:H], in_=wstage2[:, 0:H])
        nc.vector.tensor_copy(out=wm16b[:, 0:H], in_=wstage2[:, H : 2 * H])
        bi128 = const.tile([128, 2], F32)  # col 0 = bi, col 1 = bm; rows 96+ = 0
        nc.gpsimd.memset(bi128[H:128, :], 0.0)
        nc.gpsimd.tensor_copy(out=bi128[0:H, :], in_=bstage)

        # mask -> additive exp bias (mask-1)*1e30, natural layout (no transpose)
        maskf = const.tile([128, NB], F32)
        nc.vector.tensor_copy(out=maskf, in_=mask_sb)
        nc.vector.tensor_scalar(
            out=maskf, in0=maskf, scalar1=1.0, scalar2=NEG_BIG,
            op0=Sub, op1=Mult,
        )

        # mtilde fp8 U stationaries, (j, d)-contiguous so the DoubleRow
        # weight slices [:, 2jp:2jp+2, :] merge into one run (ISA req):
        # m8a = m cols 0:128; m8b = [m cols 128:150 | 0-pad | 1] (denominator
        # column lands at U2 partition 32).
        m8a = const.tile([128, NB, 128], F8)
        m8b = const.tile([128, NB, 64], F8)
        nc.gpsimd.memset(m8b[:, :, 22:32], 0.0)
        nc.gpsimd.memset(m8b[:, :, 32:33], 1.0)
        nc.gpsimd.memset(m8b[:, :, 33:64], 0.0)
        for c in range(2):
            cs = slice(c * 8, (c + 1) * 8)
            nc.vector.tensor_copy(out=m8a[:, cs, :], in_=m_nat[:, cs, 0:128])
            nc.vector.tensor_copy(out=m8b[:, cs, 0:22], in_=m_nat[:, cs, 128:D])

        # ---- fp16 casts + transposes into T layout -----------------------
        m16 = const.tile([128, NB, D], F16)
        x16 = const.tile([128, NB, D], F16)
        mT16 = const.tile([128, JM], F16)
        mT16b = const.tile([D - 128, JM], F16)
        xT16 = const.tile([128, JX], F16)
        xT16b = const.tile([D - 128, JX], F16)
        # projected activations, fp8, DoubleRow layout [64, blk, 2, cols]
        # (k-slot pair dim adjacent to cols so DR slices are mergeable)
        mp8 = const.tile([64, NB, 2, 128], F8)
        xp8 = const.tile([64, 4, 2, 512], F8)

        def tp_group(src16, blk4, dT, dTb, eng):
            # 4 block-transposes batched into one PSUM tile + one copy
            sl4 = slice(blk4 * 128, (blk4 + 4) * 128)
            t1 = psb.tile([128, 512], F16, tag="big", name="tp1")
            t2 = psb.tile([D - 128, 512], F16, tag="big", name="tp2")
            for i in range(4):
                ps = slice(i * 128, (i + 1) * 128)
                nc.tensor.transpose(t1[:, ps], src16[:, blk4 + i, 0:128], ident16)
                nc.tensor.transpose(t2[:, ps], src16[:, blk4 + i, 128:D], ident16)
            (eng.tensor_copy if eng is not nc.scalar else eng.copy)(
                out=dT[:, sl4], in_=t1
            )
            (eng.tensor_copy if eng is not nc.scalar else eng.copy)(
                out=dTb[:, sl4], in_=t2
            )

        def proj(wa, wb, bcol, srcT, srcTb, dst8, q, relu_eng):
            qs = slice(q * 512, (q + 1) * 512)
            pp = psb.tile([128, 512], F32, tag="big", name="pp")
            nc.tensor.matmul(
                pp, wa, srcT[:, qs], start=True, stop=False, skip_group_check=True
            )
            nc.tensor.matmul(
                pp, wb, srcTb[:, qs], start=False, stop=True, skip_group_check=True
            )
            for i in range(2):
                hs = slice(i * 64, (i + 1) * 64)
                if dst8 is mp8:
                    dview = dst8[:, 4 * q : 4 * q + 4, i, :]
                else:
                    dview = dst8[:, q, i, :]
                if relu_eng is nc.scalar:
                    nc.scalar.activation(
                        out=dview, in_=pp[hs, :], func=Relu,
                        bias=bi128[hs, bcol : bcol + 1], scale=1.0,
                    )
                else:
                    relu_eng.tensor_scalar(
                        out=dview, in0=pp[hs, :],
                        scalar1=bi128[hs, bcol : bcol + 1], scalar2=0.0,
                        op0=Add, op1=Max,
                    )

        # m pipeline: cast chunk -> transposes -> projection quarter
        for c4 in range(0, NB, 4):
            nc.vector.tensor_copy(
                out=m16[:, c4 : c4 + 4, :], in_=m_nat[:, c4 : c4 + 4, :]
            )
            tp_group(m16, c4, mT16, mT16b, nc.vector if c4 % 8 == 0 else nc.scalar)
            proj(wm16, wm16b, 1, mT16, mT16b, mp8, c4 // 4, nc.scalar)
        # x: cast + transpose all blocks, project only quarter 0 up front;
        # quarters 1-3 are interleaved into attention h0's PE slack.
        for c4 in range(0, NB, 4):
            nc.vector.tensor_copy(
                out=x16[:, c4 : c4 + 4, :], in_=x_nat[:, c4 : c4 + 4, :]
            )
            tp_group(x16, c4, xT16, xT16b, nc.scalar if c4 % 8 == 0 else nc.vector)
        proj(wi16, wi16b, 0, xT16, xT16b, xp8, 0, nc.vector)
        proj(wi16, wi16b, 0, xT16, xT16b, xp8, 1, nc.vector)

        # gate weights fp16 (4 g-chunks)
        wg16 = []
        for gi in range(4):
            w = const.tile(
                [wgst[gi].shape[0], G], F16, tag=f"wg16_{gi}", name=f"wg16_{gi}"
            )
            (nc.gpsimd if gi % 2 == 0 else nc.vector).tensor_copy(
                out=w, in_=wgst[gi]
            )
            wg16.append(w)
        bg_half = []
        for gi in range(3):
            t = const.tile(
                [bg_st[gi].shape[0], 1], F32, tag=f"bgh_{gi}", name=f"bgh_{gi}"
            )
            nc.gpsimd.tensor_scalar_mul(out=t, in0=bg_st[gi], scalar1=0.5)
            bg_half.append(t)

        # ---- attention h0 / h1 -------------------------------------------
        # Per half: 16 j-tiles of scores+exp, 8 DoubleRow U pairs.  The exp
        # (ACT) is the rate limiter, so leftover x projections are slotted
        # into h0's PE stream.  U tiles are copied to SBUF right after each
        # half so the single-buffer PSUM pools can be reused.
        U1c, U2c, rr16n = [], [], {}

        def emit_scores(h, j):
            sp = psb.tile([128, HALF], F32, tag="big", name="sp")
            for s in range(2):
                nc.tensor.matmul(
                    sp[:, s * 512 : (s + 1) * 512],
                    mp8[:, j], xp8[:, 2 * h + s],
                    start=True, stop=True, perf_mode=DR, skip_group_check=True,
                )
            return sp

        def emit_exp(sp, j, e8, slot):
            nc.scalar.activation(
                out=e8[:, :, slot, :], in_=sp, func=Exp,
                bias=maskf[:, j : j + 1], scale=SCALE,
            )

        def emit_U(U1, U2, e8, jp):
            first, last = jp == 0, jp == 7
            for s in range(2):
                ps = slice(s * 512, (s + 1) * 512)
                nc.tensor.matmul(
                    U1[:, ps], m8a[:, 2 * jp : 2 * jp + 2], e8[:, s],
                    start=first, stop=last, perf_mode=DR, skip_group_check=True,
                )
                nc.tensor.matmul(
                    U2[:, ps], m8b[:, 2 * jp : 2 * jp + 2], e8[:, s],
                    start=first, stop=last, perf_mode=DR, skip_group_check=True,
                )

        def emit_denomT(h):
            # denom row [1,1024] -> [128, 8] via PE transposes (so the DVE
            # reciprocal runs partition-parallel instead of 6.5ns/elem on a
            # single lane), then PE-transpose back to a [1, 1024] row.
            # The [1,1] identity sits at partition 32 to match the denom
            # row's base partition (engine APs need 32-aligned bases).
            dps = psb.tile([128, 8], F32, tag="big", name="dps")
            for c in range(8):
                nc.tensor.transpose(
                    dps[:, c : c + 1],
                    U2c[h][32:33, c * 128 : (c + 1) * 128],
                    ident32[32:33, 32:33],
                )
            rrT = work.tile([128, 8], F32, tag="rrT")
            nc.vector.reciprocal(out=rrT, in_=dps)
            rrT16 = work.tile([128, 8], F16, tag="rrT16")
            nc.vector.tensor_copy(out=rrT16, in_=rrT)
            rrps = psb.tile([1, HALF], F16, tag="big", name="rrps")
            for c in range(8):
                nc.tensor.transpose(
                    rrps[:, c * 128 : (c + 1) * 128],
                    rrT16[:, c : c + 1],
                    ident16,
                )
            rr = work.tile([1, HALF], F16, tag="rr16")
            nc.vector.tensor_copy(out=rr[:, 0:512], in_=rrps[:, 0:512])
            nc.scalar.copy(out=rr[:, 512:HALF], in_=rrps[:, 512:HALF])
            rr16n[h] = rr

        # tail stage emitters (interleaved into the next half's attention
        # stream so the PE queue never drains -- a drained PE re-throttles
        # the HAM clock gate to 1.2 GHz)
        o_re = o_d.rearrange("(q n) k -> q n k", n=NB)
        kranges = [(0, 128), (128, 256), (256, G)]
        tails = {}

        def t_bc(h):
            hs = slice(h * HALF, (h + 1) * HALF)
            st = {}
            bc = psb.tile([128, HALF], F32, tag="big", name="bc")
            for s2 in range(2):
                ps = slice(s2 * 512, (s2 + 1) * 512)
                nc.tensor.matmul(
                    bc[:, ps], ones16, rr16n[h][:, ps],
                    start=True, stop=True, skip_group_check=True,
                )
            RC16 = work.tile([128, HALF], F16, tag="RC16")
            nc.vector.tensor_mul(out=RC16, in0=U1c[h], in1=bc)
            RD16 = work.tile([D - 128, HALF], F16, tag="RD16")
            nc.vector.tensor_mul(
                out=RD16, in0=U2c[h][0 : D - 128, :], in1=bc[0 : D - 128, :]
            )
            # output-aligned res staging (partition shifts via DMA)
            res_mid = work.tile([128, HALF], F16, tag="res_mid")
            nc.vector.tensor_copy(out=res_mid[0 : D - 128, :], in_=xT16b[:, hs])
            nc.sync.dma_start(
                out=res_mid[D - 128 : 128, :], in_=RC16[0 : 256 - D, :]
            )
            res_hi = work.tile([G - 256, HALF], F16, tag="res_hi")
            nc.scalar.dma_start(
                out=res_hi[0 : D - 128, :], in_=RC16[256 - D : 128, :]
            )
            nc.sync.dma_start(out=res_hi[D - 128 : G - 256, :], in_=RD16)
            st["res_g"] = [xT16[:, hs], xT16b[:, hs], RC16, RD16]
            st["res_k"] = [xT16[:, hs], res_mid, res_hi]
            st["oT"] = [
                work.tile([128, HALF], F16, tag="oT0", name="oT0"),
                work.tile([128, HALF], F16, tag="oT1", name="oT1"),
                work.tile([G - 256, HALF], F16, tag="oT2", name="oT2"),
            ]
            st["gp"] = {}
            tails[h] = st

        def t_gate_mm(h, kc, s2):
            st = tails[h]
            k0, k1 = kranges[kc]
            if s2 == 0:
                st["gp"][kc] = psb.tile(
                    [k1 - k0, HALF], F32, tag="big", name="gp"
                )
            gp = st["gp"][kc]
            ps = slice(s2 * 512, (s2 + 1) * 512)
            for gc in range(4):
                nc.tensor.matmul(
                    gp[:, ps], wg16[gc][:, k0:k1], st["res_g"][gc][:, ps],
                    start=(gc == 0), stop=(gc == 3), skip_group_check=True,
                )

        def t_gate_act(h, kc):
            # gate = sigmoid(z) = 0.5*(1+tanh(z/2)); tanh shares the exp ACT
            # table.  (gs+1) folds into the DVE multiply, 0.5 into the final
            # PSUM->SBUF copy.
            st = tails[h]
            k0, k1 = kranges[kc]
            gs = work.tile([k1 - k0, HALF], F16, tag="gs", bufs=3)
            nc.scalar.activation(
                out=gs, in_=st["gp"][kc], func=Tanh, bias=bg_half[kc], scale=0.5
            )
            nc.vector.scalar_tensor_tensor(
                out=st["oT"][kc], in0=gs, scalar=1.0, in1=st["res_k"][kc],
                op0=Add, op1=Mult,
            )

        def t_out(h):
            # transpose back (fp16), scale by 0.5 on the upconverting copy,
            # store 4 blocks per DMA
            oT = tails[h]["oT"]
            for g4 in range(2):
                onat = work.tile([128, 4, 304], F32, tag="onat", bufs=2)
                for i in range(2):
                    op2 = psb.tile([128, 2, 304], F16, tag="big", name="op2")
                    for k in range(2):
                        n_loc = g4 * 4 + i * 2 + k
                        sl = slice(n_loc * 128, (n_loc + 1) * 128)
                        nc.tensor.transpose(
                            op2[:, k, 0:128], oT[0][:, sl], ident16
                        )
                        nc.tensor.transpose(
                            op2[:, k, 128:256], oT[1][:, sl], ident16
                        )
                        nc.tensor.transpose(
                            op2[:, k, 256:G], oT[2][:, sl],
                            ident16[: G - 256, : G - 256],
                        )
                    osl = slice(i * 2, (i + 1) * 2)
                    if i == 0:
                        nc.vector.tensor_scalar_mul(
                            out=onat[:, osl, :], in0=op2, scalar1=0.5
                        )
                    else:
                        nc.scalar.activation(
                            out=onat[:, osl, :], in_=op2, func=Copy,
                            bias=0.0, scale=0.5,
                        )
                n0 = h * 8 + g4 * 4
                dq = nc.sync if g4 % 2 == 0 else nc.scalar
                dq.dma_start(
                    out=o_re[:, n0 : n0 + 4, :], in_=onat[:, :, 0:G]
                )

        for h in range(2):
            U1 = psu.tile([128, HALF], F32, tag="u1")
            U2 = psu.tile([64, HALF], F32, tag="u2")
            sps, e8s = {}, {}
            e8s[0] = epool.tile([128, 2, 2, 512], F8, tag="e8", name="e8")
            sps[0] = emit_scores(h, 0)
            if dbg and h == 0:
                sp0dbg = const.tile([128, HALF], F32)
                nc.vector.tensor_copy(out=sp0dbg, in_=sps[0])
            sps[1] = emit_scores(h, 1)
            for jp in range(8):
                emit_exp(sps.pop(2 * jp), 2 * jp, e8s[jp], 0)
                if 2 * jp + 2 < 16:
                    sps[2 * jp + 2] = emit_scores(h, 2 * jp + 2)
                emit_exp(sps.pop(2 * jp + 1), 2 * jp + 1, e8s[jp], 1)
                if 2 * jp + 3 < 16:
                    sps[2 * jp + 3] = emit_scores(h, 2 * jp + 3)
                if jp + 1 < 8:
                    e8s[jp + 1] = epool.tile([128, 2, 2, 512], F8, tag="e8", name="e8")
                e8cur = e8s.pop(jp)
                if dbg and h == 0 and jp == 0:
                    e8dbg = const.tile([128, 2, 2, 512], F32)
                    nc.vector.tensor_copy(out=e8dbg, in_=e8cur)
                emit_U(U1, U2, e8cur, jp)
                # interleaved fill work: h0 gets the remaining x projections,
                # h1 gets h0's entire gating tail
                if h == 0 and jp in (1, 3):
                    proj(
                        wi16, wi16b, 0, xT16, xT16b, xp8,
                        2 + (jp - 1) // 2, nc.vector,
                    )
                if h == 1:
                    if jp == 0:
                        emit_denomT(0)
                    elif jp == 1:
                        t_bc(0)
                    elif jp >= 2:
                        kc, s2 = (jp - 2) // 2, jp % 2
                        t_gate_mm(0, kc, s2)
                        if s2 == 1:
                            t_gate_act(0, kc)
            u1c = work.tile([128, HALF], F32, tag="u1c")
            nc.vector.tensor_copy(out=u1c, in_=U1)
            u2c = work.tile([33, HALF], F32, tag="u2c")
            nc.scalar.copy(out=u2c, in_=U2[0:33, :])
            U1c.append(u1c)
            U2c.append(u2c)

        # ---- trailing tail: h0 stores, then all of h1's tail --------------
        emit_denomT(1)
        t_out(0)
        t_bc(1)
        for kc in range(3):
            t_gate_mm(1, kc, 0)
            t_gate_mm(1, kc, 1)
            t_gate_act(1, kc)
        t_out(1)

        if dbg:
            mpf = const.tile([64, NB, 2, 128], F32)
            nc.vector.tensor_copy(out=mpf, in_=mp8)
            nc.sync.dma_start(out=dbg["mp"][:], in_=mpf)
            xpf = const.tile([64, 4, 2, 512], F32)
            nc.vector.tensor_copy(out=xpf, in_=xp8)
            nc.sync.dma_start(out=dbg["xp"][:], in_=xpf)
            nc.sync.dma_start(out=dbg["u1"][:], in_=U1c[0])
            nc.sync.dma_start(out=dbg["u2"][:], in_=U2c[0])
            rrf = const.tile([1, HALF], F32)
            nc.vector.tensor_copy(out=rrf, in_=rr16n[0])
            nc.sync.dma_start(out=dbg["rr"][:], in_=rrf)
            nc.sync.dma_start(out=dbg["sp0"][:], in_=sp0dbg)
            nc.sync.dma_start(out=dbg["e8"][:], in_=e8dbg)
            m8af = const.tile([128, NB, 128], F32)
            nc.vector.tensor_copy(out=m8af, in_=m8a)
            nc.sync.dma_start(out=dbg["m8a"][:], in_=m8af)
            nc.sync.dma_start(out=dbg["maskf"][:], in_=maskf)


_NC_CACHE = None


def _build_nc():
    global _NC_CACHE
    if _NC_CACHE is not None:
        return _NC_CACHE
    nc = bacc.Bacc(None, target_bir_lowering=False, debug=False)
    x_d = nc.dram_tensor("x", [JX, D], F32, kind="ExternalInput")
    m_d = nc.dram_tensor("m", [JM, D], F32, kind="ExternalInput")
    mask_d = nc.dram_tensor("mask", [JM], I32, kind="ExternalInput")
    wi_d = nc.dram_tensor("Wi", [D, H], F32, kind="ExternalInput")
    bi_d = nc.dram_tensor("bi", [H], F32, kind="ExternalInput")
    wm_d = nc.dram_tensor("Wm", [D, H], F32, kind="ExternalInput")
    bm_d = nc.dram_tensor("bm", [H], F32, kind="ExternalInput")
    wg_d = nc.dram_tensor("Wg", [G, G], F32, kind="ExternalInput")
    bg_d = nc.dram_tensor("bg", [G], F32, kind="ExternalInput")
    o_d = nc.dram_tensor("out", [JX, G], F32, kind="ExternalOutput")
    dbg = None
    if DEBUG_DUMP:
        dbg = {
            "mp": nc.dram_tensor("dbg_mp", [64, NB, 2, 128], F32, kind="ExternalOutput"),
            "xp": nc.dram_tensor("dbg_xp", [64, 4, 2, 512], F32, kind="ExternalOutput"),
            "u1": nc.dram_tensor("dbg_u1", [128, HALF], F32, kind="ExternalOutput"),
            "u2": nc.dram_tensor("dbg_u2", [33, HALF], F32, kind="ExternalOutput"),
            "rr": nc.dram_tensor("dbg_rr", [1, HALF], F32, kind="ExternalOutput"),
            "sp0": nc.dram_tensor("dbg_sp0", [128, HALF], F32, kind="ExternalOutput"),
            "e8": nc.dram_tensor("dbg_e8", [128, 2, 2, 512], F32, kind="ExternalOutput"),
            "m8a": nc.dram_tensor("dbg_m8a", [128, NB, 128], F32, kind="ExternalOutput"),
            "maskf": nc.dram_tensor("dbg_maskf", [128, NB], F32, kind="ExternalOutput"),
        }
    with tile.TileContext(nc) as tc:
        _body(tc, x_d, m_d, mask_d, wi_d, bi_d, wm_d, bm_d, wg_d, bg_d, o_d,
              dbg=dbg)
    nc.finalize()
    _NC_CACHE = nc
    return nc


def _in_maps(inputs, memory, mask, Wi, bi, Wm, bm, Wg, bg):
    maps = []
    for b in range(B):
        maps.append(
            {
                "x": np.ascontiguousarray(inputs[b], dtype=np.float32),
                "m": np.ascontiguousarray(memory[b], dtype=np.float32),
                "mask": np.ascontiguousarray(mask[b], dtype=np.int32),
                "Wi": np.ascontiguousarray(Wi, dtype=np.float32),
                "bi": np.ascontiguousarray(bi, dtype=np.float32),
                "Wm": np.ascontiguousarray(Wm, dtype=np.float32),
                "bm": np.ascontiguousarray(bm, dtype=np.float32),
                "Wg": np.ascontiguousarray(Wg, dtype=np.float32),
                "bg": np.ascontiguousarray(bg, dtype=np.float32),
            }
        )
    return maps


def run_spmd(inputs, memory, mask, Wi, bi, Wm, bm, Wg, bg, **spmd_kwargs):
    """Run the kernel across 8 cores; returns the BassKernelResults."""
    nc = _build_nc()
    maps = _in_maps(
        np.asarray(inputs), np.asarray(memory), np.asarray(mask),
        np.asarray(Wi), np.asarray(bi), np.asarray(Wm), np.asarray(bm),
        np.asarray(Wg), np.asarray(bg),
    )
    return run_bass_kernel_spmd(nc, maps, list(range(B)), **spmd_kwargs)


def kernel(inputs, memory, mask, Wi, bi, Wm, bm, Wg, bg):
    res = run_spmd(inputs, memory, mask, Wi, bi, Wm, bm, Wg, bg)
    out = np.stack([res.results[b]["out"] for b in range(B)], axis=0)
    return out.astype(np.float32)


# revision 18
# speedup vs baseline: 1.2857x; 1.2613x over previous
"""Trainium2 Bass kernel for nn_DotAttention (B=8 data-parallel over 8 cores).

Per core (one batch element), using a jm/jx "permuted block layout":
column c = n*128 + q of any T-layout tile corresponds to row 16q + n of the
natural tensor (from the contiguous "(p n) d" DMA).  The permutation is
consistent across scores / exp / U / gating and undone by the output store
pattern, so it never needs an explicit fixup.

  xp = relu(x @ Wi + bi)          [2048, 96]   (fp16 matmul, DVE relu -> fp8)
  mp = relu(m @ Wm + bm)          [2048, 96]   (fp16 matmul, ACT relu -> fp8)
  S.T[jm, jx] = mp . xp           fp8 DoubleRow matmul (K = 2 x 64)
  E = exp(S.T/sqrt(96) + maskbias)  ACT, fp8 out, two j-tiles per e8 pair
  U.T[d, jx] = mtilde.T @ E       fp8 DoubleRow matmul (2 jm-tiles per inst;
                                   mtilde = [m | pad | 1], so U2 row 32 is the
                                   softmax denominator)
  attn = U / denom                denom reciprocal via PE-transpose to [128,8]
                                   (partition-parallel DVE reciprocal), PE ones
                                   broadcast back to [128, jx]
  out = sigmoid(Wg.T @ res + bg) * res
      = 0.5*(1 + tanh(z/2)) * res   tanh lives in the same ACT table set as
                                   exp, so there are no table reloads; the
                                   (gs+1)*res runs on DVE and the 0.5 folds
                                   into the final PSUM->SBUF copy.

Matmul operands are fp16 except scores/U (fp8e4 DoubleRow); PSUM accumulation
is fp32 everywhere.  The whole tail runs in fp16 (5e-4 relative error budget,
gate threshold is 2e-2).
"""

import math
import os

import numpy as np

import concourse.bass as bass
import concourse.mybir as mybir
import concourse.tile as tile
from concourse import bacc
from concourse.bass_utils import run_bass_kernel_spmd
from concourse.masks import make_identity

F32 = mybir.dt.float32
F16 = mybir.dt.float16
F8 = mybir.dt.float8e4
I32 = mybir.dt.int32
DR = mybir.MatmulPerfMode.DoubleRow

B = 8
JX = 2048
JM = 2048
D = 150
H = 96
G = 300
NB = 16  # 128-column blocks per 2048
HALF = 1024
SCALE = 1.0 / math.sqrt(float(H))
NEG_BIG = 1.0e30


DEBUG_DUMP = bool(os.environ.get("KDBG"))


def _body(tc, x_d, m_d, mask_d, wi_d, bi_d, wm_d, bm_d, wg_d, bg_d, o_d,
          dbg=None):
    nc = tc.nc
    Relu = mybir.ActivationFunctionType.Relu
    Exp = mybir.ActivationFunctionType.Exp
    Tanh = mybir.ActivationFunctionType.Tanh
    Copy = mybir.ActivationFunctionType.Copy
    Add = mybir.AluOpType.add
    Max = mybir.AluOpType.max
    Mult = mybir.AluOpType.mult
    Sub = mybir.AluOpType.subtract

    import contextlib

    with contextlib.ExitStack() as ctx:
        const = ctx.enter_context(tc.tile_pool(name="const", bufs=1))
        work = ctx.enter_context(tc.tile_pool(name="work", bufs=2))
        epool = ctx.enter_context(tc.tile_pool(name="epool", bufs=3))
        psb = ctx.enter_context(tc.tile_pool(name="psb", bufs=2, space="PSUM"))
        psu = ctx.enter_context(tc.tile_pool(name="psu", bufs=1, space="PSUM"))

        # ---- input DMAs first: they are the longest poles ----------------
        # x/m in "(p n) d" layout: 9600B contiguous per partition.
        x_nat = const.tile([128, NB, D], F32)
        m_nat = const.tile([128, NB, D], F32)
        x_re = x_d.rearrange("(p n) d -> p n d", n=NB)
        m_re = m_d.rearrange("(p n) d -> p n d", n=NB)
        for c in range(4):
            cs = slice(c * 4, (c + 1) * 4)
            nc.scalar.dma_start(out=m_nat[:, cs, :], in_=m_re[:, cs, :])
            nc.sync.dma_start(out=x_nat[:, cs, :], in_=x_re[:, cs, :])
        mask_sb = const.tile([128, NB], I32)
        wstage = const.tile([128, 2 * H], F32)
        wstage2 = const.tile([D - 128, 2 * H], F32)
        bstage = const.tile([H, 2], F32)

        # ---- constants (emitted before the gpsimd DMA dispatches so the
        # identity build isn't stuck behind ~10 serialized 0.7us dispatches)
        ident16 = const.tile([128, 128], F16)
        make_identity(nc, ident16)
        ident32 = const.tile([128, 128], F32)
        make_identity(nc, ident32)
        ones16 = const.tile([1, 128], F16)
        nc.gpsimd.memset(ones16, 1.0)
        # preload the exp ACT table while DMAs land (tanh/relu/copy share it)
        dummy = const.tile([1, 1], F32)
        nc.scalar.activation(
            out=dummy, in_=ident32[0:1, 0:1], func=Exp, scale=1.0
        )

        # small-weight DMAs on the gpsimd queue (inputs own sync/scalar)
        nc.gpsimd.dma_start(out=wstage[:, 0:H], in_=wi_d[0:128, :])
        nc.gpsimd.dma_start(out=wstage[:, H : 2 * H], in_=wm_d[0:128, :])
        nc.gpsimd.dma_start(out=wstage2[:, 0:H], in_=wi_d[128:D, :])
        nc.gpsimd.dma_start(out=wstage2[:, H : 2 * H], in_=wm_d[128:D, :])
        nc.gpsimd.dma_start(
            out=bstage[:, 0:1], in_=bi_d.rearrange("(n one) -> n one", one=1)
        )
        nc.gpsimd.dma_start(
            out=bstage[:, 1:2], in_=bm_d.rearrange("(n one) -> n one", one=1)
        )
        nc.gpsimd.dma_start(
            out=mask_sb, in_=mask_d.rearrange("(p n) -> p n", n=NB)
        )
        # Wg / bg are needed only ~40us in; queue them behind the inputs
        wgst = []
        for gi, (g0, g1) in enumerate([(0, 128), (128, D), (D, D + 128), (D + 128, G)]):
            wst = const.tile([g1 - g0, G], F32, tag=f"wgst_{gi}", name=f"wgst_{gi}")
            (nc.sync if gi % 2 == 0 else nc.scalar).dma_start(
                out=wst, in_=wg_d[g0:g1, :]
            )
            wgst.append(wst)
        bg_st = []
        for gi, (g0, g1) in enumerate([(0, 128), (128, 256), (256, G)]):
            t = const.tile([g1 - g0, 1], F32, tag=f"bgst_{gi}", name=f"bgst_{gi}")
            nc.scalar.dma_start(
                out=t, in_=bg_d[g0:g1].rearrange("(n one) -> n one", one=1)
            )
            bg_st.append(t)

        # ---- weight prep -------------------------------------------------
        wi16 = const.tile([128, 128], F16)  # cols 96:128 zero-padded
        wm16 = const.tile([128, 128], F16)
        nc.gpsimd.memset(wi16[:, H:128], 0.0)
        nc.gpsimd.memset(wm16[:, H:128], 0.0)
        nc.vector.tensor_copy(out=wi16[:, 0:H], in_=wstage[:, 0:H])
        nc.vector.tensor_copy(out=wm16[:, 0:H], in_=wstage[:, H : 2 * H])
        wi16b = const.tile([D - 128, 128], F16)
        wm16b = const.tile([D - 128, 128], F16)
        nc.gpsimd.memset(wi16b[:, H:128], 0.0)
        nc.gpsimd.memset(wm16b[:, H:128], 0.0)
        nc.vector.tensor_copy(out=wi16b[:, 0:H], in_=wstage2[:, 0:H])
        nc.vector.tensor_copy(out=wm16b[:, 0:H], in_=wstage2[:, H : 2 * H])
        bi128 = const.tile([128, 2], F32)  # col 0 = bi, col 1 = bm; rows 96+ = 0
        nc.gpsimd.memset(bi128[H:128, :], 0.0)
        nc.gpsimd.tensor_copy(out=bi128[0:H, :], in_=bstage)

        # mask -> additive exp bias (mask-1)*1e30, natural layout (no transpose)
        maskf = const.tile([128, NB], F32)
        nc.vector.tensor_copy(out=maskf, in_=mask_sb)
        nc.vector.tensor_scalar(
            out=maskf, in0=maskf, scalar1=1.0, scalar2=NEG_BIG,
            op0=Sub, op1=Mult,
        )

        # mtilde fp8 U stationaries, (j, d)-contiguous so the DoubleRow
        # weight slices [:, 2jp:2jp+2, :] merge into one run (ISA req):
        # m8a = m cols 0:128; m8b = [m cols 128:150 | 0-pad | 1] (denominator
        # column lands at U2 partition 32).
        m8a = const.tile([128, NB, 128], F8)
        m8b = const.tile([128, NB, 64], F8)
        nc.gpsimd.memset(m8b[:, :, 22:32], 0.0)
        nc.gpsimd.memset(m8b[:, :, 32:33], 1.0)
        nc.gpsimd.memset(m8b[:, :, 33:64], 0.0)
        for c in range(2):
            cs = slice(c * 8, (c + 1) * 8)
            nc.vector.tensor_copy(out=m8a[:, cs, :], in_=m_nat[:, cs, 0:128])
            nc.vector.tensor_copy(out=m8b[:, cs, 0:22], in_=m_nat[:, cs, 128:D])

        # ---- fp16 casts + transposes into T layout -----------------------
        m16 = const.tile([128, NB, D], F16)
        x16 = const.tile([128, NB, D], F16)
        mT16 = const.tile([128, JM], F16)
        mT16b = const.tile([D - 128, JM], F16)
        xT16 = const.tile([128, JX], F16)
        xT16b = const.tile([D - 128, JX], F16)
        # projected activations, fp16 T layout (scores run as plain fp16
        # matmuls: DoubleRow streams both k-planes serially, so DR would
        # DOUBLE the score cost vs one K=96 fp16 pass)
        mpT16 = const.tile([96, JM], F16)
        xpT16 = const.tile([96, JX], F16)

        def tp_group(src16, blk4, dT, dTb, eng):
            # 4 block-transposes batched into one PSUM tile + one copy
            sl4 = slice(blk4 * 128, (blk4 + 4) * 128)
            t1 = psb.tile([128, 512], F16, tag="big", name="tp1")
            t2 = psb.tile([D - 128, 512], F16, tag="big", name="tp2")
            for i in range(4):
                ps = slice(i * 128, (i + 1) * 128)
                nc.tensor.transpose(t1[:, ps], src16[:, blk4 + i, 0:128], ident16)
                nc.tensor.transpose(t2[:, ps], src16[:, blk4 + i, 128:D], ident16)
            (eng.tensor_copy if eng is not nc.scalar else eng.copy)(
                out=dT[:, sl4], in_=t1
            )
            (eng.tensor_copy if eng is not nc.scalar else eng.copy)(
                out=dTb[:, sl4], in_=t2
            )

        def proj(wa, wb, bcol, srcT, srcTb, dst8, q, relu_eng):
            qs = slice(q * 512, (q + 1) * 512)
            pp = psb.tile([128, 512], F32, tag="big", name="pp")
            nc.tensor.matmul(
                pp, wa, srcT[:, qs], start=True, stop=False, skip_group_check=True
            )
            nc.tensor.matmul(
                pp, wb, srcTb[:, qs], start=False, stop=True, skip_group_check=True
            )
            if relu_eng is nc.scalar:
                nc.scalar.activation(
                    out=dst8[:, qs], in_=pp[0:H, :], func=Relu,
                    bias=bi128[0:H, bcol : bcol + 1], scale=1.0,
                )
            else:
                relu_eng.tensor_scalar(
                    out=dst8[:, qs], in0=pp[0:H, :],
                    scalar1=bi128[0:H, bcol : bcol + 1], scalar2=0.0,
                    op0=Add, op1=Max,
                )

        # m pipeline: cast chunk -> transposes -> projection quarter
        for c4 in range(0, NB, 4):
            nc.vector.tensor_copy(
                out=m16[:, c4 : c4 + 4, :], in_=m_nat[:, c4 : c4 + 4, :]
            )
            tp_group(m16, c4, mT16, mT16b, nc.vector if c4 % 8 == 0 else nc.scalar)
            proj(wm16, wm16b, 1, mT16, mT16b, mpT16, c4 // 4, nc.scalar)
        # x: cast + transpose all blocks, project only quarter 0 up front;
        # quarters 1-3 are interleaved into attention h0's PE slack.
        for c4 in range(0, NB, 4):
            nc.vector.tensor_copy(
                out=x16[:, c4 : c4 + 4, :], in_=x_nat[:, c4 : c4 + 4, :]
            )
            tp_group(x16, c4, xT16, xT16b, nc.scalar if c4 % 8 == 0 else nc.vector)
        proj(wi16, wi16b, 0, xT16, xT16b, xpT16, 0, nc.vector)
        proj(wi16, wi16b, 0, xT16, xT16b, xpT16, 1, nc.vector)

        # gate weights fp16 (4 g-chunks)
        wg16 = []
        for gi in range(4):
            w = const.tile(
                [wgst[gi].shape[0], G], F16, tag=f"wg16_{gi}", name=f"wg16_{gi}"
            )
            (nc.gpsimd if gi % 2 == 0 else nc.vector).tensor_copy(
                out=w, in_=wgst[gi]
            )
            wg16.append(w)
        bg_half = []
        for gi in range(3):
            t = const.tile(
                [bg_st[gi].shape[0], 1], F32, tag=f"bgh_{gi}", name=f"bgh_{gi}"
            )
            nc.gpsimd.tensor_scalar_mul(out=t, in0=bg_st[gi], scalar1=0.5)
            bg_half.append(t)

        # ---- attention h0 / h1 -------------------------------------------
        # Per half: 16 j-tiles of scores+exp, 8 DoubleRow U pairs.  The exp
        # (ACT) is the rate limiter, so leftover x projections are slotted
        # into h0's PE stream.  U tiles are copied to SBUF right after each
        # half so the single-buffer PSUM pools can be reused.
        U1c, U2c, rr16n = [], [], {}

        def emit_scores(h, j):
            sp = psb.tile([128, HALF], F32, tag="big", name="sp")
            for s in range(2):
                ss = slice(h * HALF + s * 512, h * HALF + (s + 1) * 512)
                nc.tensor.matmul(
                    sp[:, s * 512 : (s + 1) * 512],
                    mpT16[:, j * 128 : (j + 1) * 128], xpT16[:, ss],
                    start=True, stop=True, skip_group_check=True,
                )
            return sp

        def emit_exp(sp, j, e8, slot):
            nc.scalar.activation(
                out=e8[:, :, slot, :], in_=sp, func=Exp,
                bias=maskf[:, j : j + 1], scale=SCALE,
            )

        def emit_U(U1, U2, e8, jp):
            first, last = jp == 0, jp == 7
            for s in range(2):
                ps = slice(s * 512, (s + 1) * 512)
                nc.tensor.matmul(
                    U1[:, ps], m8a[:, 2 * jp : 2 * jp + 2], e8[:, s],
                    start=first, stop=last, perf_mode=DR, skip_group_check=True,
                )
                nc.tensor.matmul(
                    U2[:, ps], m8b[:, 2 * jp : 2 * jp + 2], e8[:, s],
                    start=first, stop=last, perf_mode=DR, skip_group_check=True,
                )

        def emit_denomT(h):
            # denom row [1,1024] -> [128, 8] via PE transposes (so the DVE
            # reciprocal runs partition-parallel instead of 6.5ns/elem on a
            # single lane), then PE-transpose back to a [1, 1024] row.
            # The [1,1] identity sits at partition 32 to match the denom
            # row's base partition (engine APs need 32-aligned bases).
            dps = psb.tile([128, 8], F32, tag="big", name="dps")
            for c in range(8):
                nc.tensor.transpose(
                    dps[:, c : c + 1],
                    U2c[h][32:33, c * 128 : (c + 1) * 128],
                    ident32[32:33, 32:33],
                )
            rrT = work.tile([128, 8], F32, tag="rrT")
            nc.vector.reciprocal(out=rrT, in_=dps)
            rrT16 = work.tile([128, 8], F16, tag="rrT16")
            nc.vector.tensor_copy(out=rrT16, in_=rrT)
            rrps = psb.tile([1, HALF], F16, tag="big", name="rrps")
            for c in range(8):
                nc.tensor.transpose(
                    rrps[:, c * 128 : (c + 1) * 128],
                    rrT16[:, c : c + 1],
                    ident16,
                )
            rr = work.tile([1, HALF], F16, tag="rr16")
            nc.vector.tensor_copy(out=rr[:, 0:512], in_=rrps[:, 0:512])
            nc.scalar.copy(out=rr[:, 512:HALF], in_=rrps[:, 512:HALF])
            rr16n[h] = rr

        # tail stage emitters (interleaved into the next half's attention
        # stream so the PE queue never drains -- a drained PE re-throttles
        # the HAM clock gate to 1.2 GHz)
        o_re = o_d.rearrange("(q n) k -> q n k", n=NB)
        kranges = [(0, 128), (128, 256), (256, G)]
        tails = {}

        def t_bc(h):
            hs = slice(h * HALF, (h + 1) * HALF)
            st = {}
            bc = psb.tile([128, HALF], F32, tag="big", name="bc")
            for s2 in range(2):
                ps = slice(s2 * 512, (s2 + 1) * 512)
                nc.tensor.matmul(
                    bc[:, ps], ones16, rr16n[h][:, ps],
                    start=True, stop=True, skip_group_check=True,
                )
            RC16 = work.tile([128, HALF], F16, tag="RC16")
            nc.vector.tensor_mul(out=RC16, in0=U1c[h], in1=bc)
            RD16 = work.tile([D - 128, HALF], F16, tag="RD16")
            nc.vector.tensor_mul(
                out=RD16, in0=U2c[h][0 : D - 128, :], in1=bc[0 : D - 128, :]
            )
            # output-aligned res staging (partition shifts via DMA)
            res_mid = work.tile([128, HALF], F16, tag="res_mid")
            nc.vector.tensor_copy(out=res_mid[0 : D - 128, :], in_=xT16b[:, hs])
            nc.sync.dma_start(
                out=res_mid[D - 128 : 128, :], in_=RC16[0 : 256 - D, :]
            )
            res_hi = work.tile([G - 256, HALF], F16, tag="res_hi")
            nc.scalar.dma_start(
                out=res_hi[0 : D - 128, :], in_=RC16[256 - D : 128, :]
            )
            nc.sync.dma_start(out=res_hi[D - 128 : G - 256, :], in_=RD16)
            st["res_g"] = [xT16[:, hs], xT16b[:, hs], RC16, RD16]
            st["res_k"] = [xT16[:, hs], res_mid, res_hi]
            st["oT"] = [
                work.tile([128, HALF], F16, tag="oT0", name="oT0"),
                work.tile([128, HALF], F16, tag="oT1", name="oT1"),
                work.tile([G - 256, HALF], F16, tag="oT2", name="oT2"),
            ]
            st["gp"] = {}
            tails[h] = st

        def t_gate_mm(h, kc, s2):
            st = tails[h]
            k0, k1 = kranges[kc]
            if s2 == 0:
                st["gp"][kc] = psb.tile(
                    [k1 - k0, HALF], F32, tag="big", name="gp"
                )
            gp = st["gp"][kc]
            ps = slice(s2 * 512, (s2 + 1) * 512)
            for gc in range(4):
                nc.tensor.matmul(
                    gp[:, ps], wg16[gc][:, k0:k1], st["res_g"][gc][:, ps],
                    start=(gc == 0), stop=(gc == 3), skip_group_check=True,
                )

        def t_gate_act(h, kc):
            # gate = sigmoid(z) = 0.5*(1+tanh(z/2)); tanh shares the exp ACT
            # table.  (gs+1) folds into the DVE multiply, 0.5 into the final
            # PSUM->SBUF copy.
            st = tails[h]
            k0, k1 = kranges[kc]
            gs = work.tile([k1 - k0, HALF], F16, tag="gs", bufs=3)
            nc.scalar.activation(
                out=gs, in_=st["gp"][kc], func=Tanh, bias=bg_half[kc], scale=0.5
            )
            nc.vector.scalar_tensor_tensor(
                out=st["oT"][kc], in0=gs, scalar=1.0, in1=st["res_k"][kc],
                op0=Add, op1=Mult,
            )

        def t_out(h):
            # transpose back (fp16), scale by 0.5 on the upconverting copy,
            # store 4 blocks per DMA
            oT = tails[h]["oT"]
            for g4 in range(2):
                onat = work.tile([128, 4, 304], F32, tag="onat", bufs=2)
                for i in range(2):
                    op2 = psb.tile([128, 2, 304], F16, tag="big", name="op2")
                    for k in range(2):
                        n_loc = g4 * 4 + i * 2 + k
                        sl = slice(n_loc * 128, (n_loc + 1) * 128)
                        nc.tensor.transpose(
                            op2[:, k, 0:128], oT[0][:, sl], ident16
                        )
                        nc.tensor.transpose(
                            op2[:, k, 128:256], oT[1][:, sl], ident16
                        )
                        nc.tensor.transpose(
                            op2[:, k, 256:G], oT[2][:, sl],
                            ident16[: G - 256, : G - 256],
                        )
                    osl = slice(i * 2, (i + 1) * 2)
                    if i == 0:
                        nc.vector.tensor_scalar_mul(
                            out=onat[:, osl, :], in0=op2, scalar1=0.5
                        )
                    else:
                        nc.scalar.activation(
                            out=onat[:, osl, :], in_=op2, func=Copy,
                            bias=0.0, scale=0.5,
                        )
                n0 = h * 8 + g4 * 4
                dq = nc.sync if g4 % 2 == 0 else nc.scalar
                dq.dma_start(
                    out=o_re[:, n0 : n0 + 4, :], in_=onat[:, :, 0:G]
                )

        for h in range(2):
            U1 = psu.tile([128, HALF], F32, tag="u1")
            U2 = psu.tile([64, HALF], F32, tag="u2")
            sps, e8s = {}, {}
            e8s[0] = epool.tile([128, 2, 2, 512], F8, tag="e8", name="e8")
            sps[0] = emit_scores(h, 0)
            if dbg and h == 0:
                sp0dbg = const.tile([128, HALF], F32)
                nc.vector.tensor_copy(out=sp0dbg, in_=sps[0])
            sps[1] = emit_scores(h, 1)
            for jp in range(8):
                emit_exp(sps.pop(2 * jp), 2 * jp, e8s[jp], 0)
                if 2 * jp + 2 < 16:
                    sps[2 * jp + 2] = emit_scores(h, 2 * jp + 2)
                emit_exp(sps.pop(2 * jp + 1), 2 * jp + 1, e8s[jp], 1)
                if 2 * jp + 3 < 16:
                    sps[2 * jp + 3] = emit_scores(h, 2 * jp + 3)
                if jp + 1 < 8:
                    e8s[jp + 1] = epool.tile([128, 2, 2, 512], F8, tag="e8", name="e8")
                e8cur = e8s.pop(jp)
                if dbg and h == 0 and jp == 0:
                    e8dbg = const.tile([128, 2, 2, 512], F32)
                    nc.vector.tensor_copy(out=e8dbg, in_=e8cur)
                emit_U(U1, U2, e8cur, jp)
                # interleaved fill work: h0 gets the remaining x projections,
                # h1 gets h0's entire gating tail
                if h == 0 and jp in (1, 3):
                    proj(
                        wi16, wi16b, 0, xT16, xT16b, xpT16,
                        2 + (jp - 1) // 2, nc.vector,
                    )
                if h == 1:
                    if jp == 0:
                        emit_denomT(0)
                    elif jp == 1:
                        t_bc(0)
                    elif jp >= 2:
                        kc, s2 = (jp - 2) // 2, jp % 2
                        t_gate_mm(0, kc, s2)
                        if s2 == 1:
                            t_gate_act(0, kc)
            u1c = work.tile([128, HALF], F32, tag="u1c")
            nc.vector.tensor_copy(out=u1c, in_=U1)
            u2c = work.tile([33, HALF], F32, tag="u2c")
            nc.scalar.copy(out=u2c, in_=U2[0:33, :])
            U1c.append(u1c)
            U2c.append(u2c)

        # ---- trailing tail: h0 stores, then all of h1's tail --------------
        emit_denomT(1)
        t_out(0)
        t_bc(1)
        for kc in range(3):
            t_gate_mm(1, kc, 0)
            t_gate_mm(1, kc, 1)
            t_gate_act(1, kc)
        t_out(1)

        if dbg:
            nc.sync.dma_start(out=dbg["u1"][:], in_=U1c[0])
            nc.sync.dma_start(out=dbg["u2"][:], in_=U2c[0])
            rrf = const.tile([1, HALF], F32)
            nc.vector.tensor_copy(out=rrf, in_=rr16n[0])
            nc.sync.dma_start(out=dbg["rr"][:], in_=rrf)
            nc.sync.dma_start(out=dbg["sp0"][:], in_=sp0dbg)
            nc.sync.dma_start(out=dbg["e8"][:], in_=e8dbg)
            m8af = const.tile([128, NB, 128], F32)
            nc.vector.tensor_copy(out=m8af, in_=m8a)
            nc.sync.dma_start(out=dbg["m8a"][:], in_=m8af)
            nc.sync.dma_start(out=dbg["maskf"][:], in_=maskf)


_NC_CACHE = None


def _build_nc():
    global _NC_CACHE
    if _NC_CACHE is not None:
        return _NC_CACHE
    nc = bacc.Bacc(None, target_bir_lowering=False, debug=False)
    x_d = nc.dram_tensor("x", [JX, D], F32, kind="ExternalInput")
    m_d = nc.dram_tensor("m", [JM, D], F32, kind="ExternalInput")
    mask_d = nc.dram_tensor("mask", [JM], I32, kind="ExternalInput")
    wi_d = nc.dram_tensor("Wi", [D, H], F32, kind="ExternalInput")
    bi_d = nc.dram_tensor("bi", [H], F32, kind="ExternalInput")
    wm_d = nc.dram_tensor("Wm", [D, H], F32, kind="ExternalInput")
    bm_d = nc.dram_tensor("bm", [H], F32, kind="ExternalInput")
    wg_d = nc.dram_tensor("Wg", [G, G], F32, kind="ExternalInput")
    bg_d = nc.dram_tensor("bg", [G], F32, kind="ExternalInput")
    o_d = nc.dram_tensor("out", [JX, G], F32, kind="ExternalOutput")
    dbg = None
    if DEBUG_DUMP:
        dbg = {
            "u1": nc.dram_tensor("dbg_u1", [128, HALF], F32, kind="ExternalOutput"),
            "u2": nc.dram_tensor("dbg_u2", [33, HALF], F32, kind="ExternalOutput"),
            "rr": nc.dram_tensor("dbg_rr", [1, HALF], F32, kind="ExternalOutput"),
            "sp0": nc.dram_tensor("dbg_sp0", [128, HALF], F32, kind="ExternalOutput"),
            "e8": nc.dram_tensor("dbg_e8", [128, 2, 2, 512], F32, kind="ExternalOutput"),
            "m8a": nc.dram_tensor("dbg_m8a", [128, NB, 128], F32, kind="ExternalOutput"),
            "maskf": nc.dram_tensor("dbg_maskf", [128, NB], F32, kind="ExternalOutput"),
        }
    with tile.TileContext(nc) as tc:
        _body(tc, x_d, m_d, mask_d, wi_d, bi_d, wm_d, bm_d, wg_d, bg_d, o_d,
              dbg=dbg)
    nc.finalize()
    _NC_CACHE = nc
    return nc


def _in_maps(inputs, memory, mask, Wi, bi, Wm, bm, Wg, bg):
    maps = []
    for b in range(B):
        maps.append(
            {
                "x": np.ascontiguousarray(inputs[b], dtype=np.float32),
                "m": np.ascontiguousarray(memory[b], dtype=np.float32),
                "mask": np.ascontiguousarray(mask[b], dtype=np.int32),
                "Wi": np.ascontiguousarray(Wi, dtype=np.float32),
                "bi": np.ascontiguousarray(bi, dtype=np.float32),
                "Wm": np.ascontiguousarray(Wm, dtype=np.float32),
                "bm": np.ascontiguousarray(bm, dtype=np.float32),
                "Wg": np.ascontiguousarray(Wg, dtype=np.float32),
                "bg": np.ascontiguousarray(bg, dtype=np.float32),
            }
        )
    return maps


def run_spmd(inputs, memory, mask, Wi, bi, Wm, bm, Wg, bg, **spmd_kwargs):
    """Run the kernel across 8 cores; returns the BassKernelResults."""
    nc = _build_nc()
    maps = _in_maps(
        np.asarray(inputs), np.asarray(memory), np.asarray(mask),
        np.asarray(Wi), np.asarray(bi), np.asarray(Wm), np.asarray(bm),
        np.asarray(Wg), np.asarray(bg),
    )
    return run_bass_kernel_spmd(nc, maps, list(range(B)), **spmd_kwargs)


def kernel(inputs, memory, mask, Wi, bi, Wm, bm, Wg, bg):
    res = run_spmd(inputs, memory, mask, Wi, bi, Wm, bm, Wg, bg)
    out = np.stack([res.results[b]["out"] for b in range(B)], axis=0)
    return out.astype(np.float32)


# revision 19
# speedup vs baseline: 1.3092x; 1.0183x over previous
"""Trainium2 Bass kernel for nn_DotAttention (B=8 data-parallel over 8 cores).

Per core (one batch element), using a jm/jx "permuted block layout":
column c = n*128 + q of any T-layout tile corresponds to row 16q + n of the
natural tensor (from the contiguous "(p n) d" DMA).  The permutation is
consistent across scores / exp / U / gating and undone by the output store
pattern, so it never needs an explicit fixup.

  xp = relu(x @ Wi + bi)          [2048, 96]   (fp16 matmul, DVE relu -> fp8)
  mp = relu(m @ Wm + bm)          [2048, 96]   (fp16 matmul, ACT relu -> fp8)
  S.T[jm, jx] = mp . xp           fp8 DoubleRow matmul (K = 2 x 64)
  E = exp(S.T/sqrt(96) + maskbias)  ACT, fp8 out, two j-tiles per e8 pair
  U.T[d, jx] = mtilde.T @ E       fp8 DoubleRow matmul (2 jm-tiles per inst;
                                   mtilde = [m | pad | 1], so U2 row 32 is the
                                   softmax denominator)
  attn = U / denom                denom reciprocal via PE-transpose to [128,8]
                                   (partition-parallel DVE reciprocal), PE ones
                                   broadcast back to [128, jx]
  out = sigmoid(Wg.T @ res + bg) * res
      = 0.5*(1 + tanh(z/2)) * res   tanh lives in the same ACT table set as
                                   exp, so there are no table reloads; the
                                   (gs+1)*res runs on DVE and the 0.5 folds
                                   into the final PSUM->SBUF copy.

Matmul operands are fp16 except scores/U (fp8e4 DoubleRow); PSUM accumulation
is fp32 everywhere.  The whole tail runs in fp16 (5e-4 relative error budget,
gate threshold is 2e-2).
"""

import math
import os

import numpy as np

import concourse.bass as bass
import concourse.mybir as mybir
import concourse.tile as tile
from concourse import bacc
from concourse.bass_utils import run_bass_kernel_spmd
from concourse.masks import make_identity

F32 = mybir.dt.float32
F16 = mybir.dt.float16
F8 = mybir.dt.float8e4
I32 = mybir.dt.int32
DR = mybir.MatmulPerfMode.DoubleRow

B = 8
JX = 2048
JM = 2048
D = 150
H = 96
G = 300
NB = 16  # 128-column blocks per 2048
HALF = 1024
SCALE = 1.0 / math.sqrt(float(H))
NEG_BIG = 1.0e30


DEBUG_DUMP = bool(os.environ.get("KDBG"))


def _body(tc, x_d, m_d, mask_d, wi_d, bi_d, wm_d, bm_d, wg_d, bg_d, o_d,
          dbg=None):
    nc = tc.nc
    Relu = mybir.ActivationFunctionType.Relu
    Exp = mybir.ActivationFunctionType.Exp
    Tanh = mybir.ActivationFunctionType.Tanh
    Copy = mybir.ActivationFunctionType.Copy
    Add = mybir.AluOpType.add
    Max = mybir.AluOpType.max
    Mult = mybir.AluOpType.mult
    Sub = mybir.AluOpType.subtract

    import contextlib

    with contextlib.ExitStack() as ctx:
        const = ctx.enter_context(tc.tile_pool(name="const", bufs=1))
        work = ctx.enter_context(tc.tile_pool(name="work", bufs=2))
        epool = ctx.enter_context(tc.tile_pool(name="epool", bufs=3))
        psb = ctx.enter_context(tc.tile_pool(name="psb", bufs=2, space="PSUM"))
        psu = ctx.enter_context(tc.tile_pool(name="psu", bufs=1, space="PSUM"))

        # ---- input DMAs first: they are the longest poles ----------------
        # x/m in "(p n) d" layout: 9600B contiguous per partition.
        x_nat = const.tile([128, NB, D], F32)
        m_nat = const.tile([128, NB, D], F32)
        x_re = x_d.rearrange("(p n) d -> p n d", n=NB)
        m_re = m_d.rearrange("(p n) d -> p n d", n=NB)
        for c in range(4):
            cs = slice(c * 4, (c + 1) * 4)
            nc.scalar.dma_start(out=m_nat[:, cs, :], in_=m_re[:, cs, :])
            nc.sync.dma_start(out=x_nat[:, cs, :], in_=x_re[:, cs, :])
        mask_sb = const.tile([128, NB], I32)
        wstage = const.tile([128, 2 * H], F32)
        wstage2 = const.tile([D - 128, 2 * H], F32)
        bstage = const.tile([H, 2], F32)

        # ---- constants (emitted before the gpsimd DMA dispatches so the
        # identity build isn't stuck behind ~10 serialized 0.7us dispatches)
        ident16 = const.tile([128, 128], F16)
        make_identity(nc, ident16)
        ident32 = const.tile([128, 128], F32)
        make_identity(nc, ident32)
        ones16 = const.tile([1, 128], F16)
        nc.gpsimd.memset(ones16, 1.0)
        # preload the exp ACT table while DMAs land (tanh/relu/copy share it)
        dummy = const.tile([1, 1], F32)
        nc.scalar.activation(
            out=dummy, in_=ident32[0:1, 0:1], func=Exp, scale=1.0
        )

        # small-weight DMAs on the gpsimd queue (inputs own sync/scalar);
        # the wi/wm halves the m-projection needs first go on gpsimd FIRST
        # so they land before the m transposes complete
        nc.gpsimd.dma_start(out=wstage[:, 0:H], in_=wi_d[0:128, :])
        nc.gpsimd.dma_start(out=wstage[:, H : 2 * H], in_=wm_d[0:128, :])
        nc.gpsimd.dma_start(out=wstage2[:, 0:H], in_=wi_d[128:D, :])
        nc.gpsimd.dma_start(out=wstage2[:, H : 2 * H], in_=wm_d[128:D, :])
        nc.gpsimd.dma_start(
            out=bstage[:, 0:1], in_=bi_d.rearrange("(n one) -> n one", one=1)
        )
        nc.gpsimd.dma_start(
            out=bstage[:, 1:2], in_=bm_d.rearrange("(n one) -> n one", one=1)
        )
        nc.gpsimd.dma_start(
            out=mask_sb, in_=mask_d.rearrange("(p n) -> p n", n=NB)
        )
        # Wg / bg are needed only ~40us in; queue them behind the inputs
        wgst = []
        for gi, (g0, g1) in enumerate([(0, 128), (128, D), (D, D + 128), (D + 128, G)]):
            wst = const.tile([g1 - g0, G], F32, tag=f"wgst_{gi}", name=f"wgst_{gi}")
            (nc.sync if gi % 2 == 0 else nc.scalar).dma_start(
                out=wst, in_=wg_d[g0:g1, :]
            )
            wgst.append(wst)
        bg_st = []
        for gi, (g0, g1) in enumerate([(0, 128), (128, 256), (256, G)]):
            t = const.tile([g1 - g0, 1], F32, tag=f"bgst_{gi}", name=f"bgst_{gi}")
            nc.scalar.dma_start(
                out=t, in_=bg_d[g0:g1].rearrange("(n one) -> n one", one=1)
            )
            bg_st.append(t)

        # ---- weight prep -------------------------------------------------
        wi16 = const.tile([128, 128], F16)  # cols 96:128 zero-padded
        wm16 = const.tile([128, 128], F16)
        nc.gpsimd.memset(wi16[:, H:128], 0.0)
        nc.gpsimd.memset(wm16[:, H:128], 0.0)
        nc.vector.tensor_copy(out=wi16[:, 0:H], in_=wstage[:, 0:H])
        nc.vector.tensor_copy(out=wm16[:, 0:H], in_=wstage[:, H : 2 * H])
        wi16b = const.tile([D - 128, 128], F16)
        wm16b = const.tile([D - 128, 128], F16)
        nc.gpsimd.memset(wi16b[:, H:128], 0.0)
        nc.gpsimd.memset(wm16b[:, H:128], 0.0)
        nc.vector.tensor_copy(out=wi16b[:, 0:H], in_=wstage2[:, 0:H])
        nc.vector.tensor_copy(out=wm16b[:, 0:H], in_=wstage2[:, H : 2 * H])
        bi128 = const.tile([128, 2], F32)  # col 0 = bi, col 1 = bm; rows 96+ = 0
        nc.gpsimd.memset(bi128[H:128, :], 0.0)
        nc.gpsimd.tensor_copy(out=bi128[0:H, :], in_=bstage)

        # mask -> additive exp bias (mask-1)*1e30, natural layout (no transpose)
        maskf = const.tile([128, NB], F32)
        nc.vector.tensor_copy(out=maskf, in_=mask_sb)
        nc.vector.tensor_scalar(
            out=maskf, in0=maskf, scalar1=1.0, scalar2=NEG_BIG,
            op0=Sub, op1=Mult,
        )

        # mtilde fp8 U stationaries, (j, d)-contiguous so the DoubleRow
        # weight slices [:, 2jp:2jp+2, :] merge into one run (ISA req):
        # m8a = m cols 0:128; m8b = [m cols 128:150 | 0-pad | 1] (denominator
        # column lands at U2 partition 32).
        m8a = const.tile([128, NB, 128], F8)
        m8b = const.tile([128, NB, 64], F8)
        nc.gpsimd.memset(m8b[:, :, 22:32], 0.0)
        nc.gpsimd.memset(m8b[:, :, 32:33], 1.0)
        nc.gpsimd.memset(m8b[:, :, 33:64], 0.0)
        for c in range(2):
            cs = slice(c * 8, (c + 1) * 8)
            nc.vector.tensor_copy(out=m8a[:, cs, :], in_=m_nat[:, cs, 0:128])
            nc.vector.tensor_copy(out=m8b[:, cs, 0:22], in_=m_nat[:, cs, 128:D])

        # ---- fp16 casts + transposes into T layout -----------------------
        m16 = const.tile([128, NB, D], F16)
        x16 = const.tile([128, NB, D], F16)
        mT16 = const.tile([128, JM], F16)
        mT16b = const.tile([D - 128, JM], F16)
        xT16 = const.tile([128, JX], F16)
        xT16b = const.tile([D - 128, JX], F16)
        # projected activations, fp16 T layout (scores run as plain fp16
        # matmuls: DoubleRow streams both k-planes serially, so DR would
        # DOUBLE the score cost vs one K=96 fp16 pass)
        mpT16 = const.tile([96, JM], F16)
        xpT16 = const.tile([96, JX], F16)

        def tp_group(src16, blk4, dT, dTb, eng):
            # 4 block-transposes batched into one PSUM tile + one copy
            sl4 = slice(blk4 * 128, (blk4 + 4) * 128)
            t1 = psb.tile([128, 512], F16, tag="big", name="tp1")
            t2 = psb.tile([D - 128, 512], F16, tag="big", name="tp2")
            for i in range(4):
                ps = slice(i * 128, (i + 1) * 128)
                nc.tensor.transpose(t1[:, ps], src16[:, blk4 + i, 0:128], ident16)
                nc.tensor.transpose(t2[:, ps], src16[:, blk4 + i, 128:D], ident16)
            (eng.tensor_copy if eng is not nc.scalar else eng.copy)(
                out=dT[:, sl4], in_=t1
            )
            (eng.tensor_copy if eng is not nc.scalar else eng.copy)(
                out=dTb[:, sl4], in_=t2
            )

        def proj(wa, wb, bcol, srcT, srcTb, dst8, q, relu_eng):
            qs = slice(q * 512, (q + 1) * 512)
            pp = psb.tile([128, 512], F32, tag="big", name="pp")
            nc.tensor.matmul(
                pp, wa, srcT[:, qs], start=True, stop=False, skip_group_check=True
            )
            nc.tensor.matmul(
                pp, wb, srcTb[:, qs], start=False, stop=True, skip_group_check=True
            )
            if relu_eng is nc.scalar:
                nc.scalar.activation(
                    out=dst8[:, qs], in_=pp[0:H, :], func=Relu,
                    bias=bi128[0:H, bcol : bcol + 1], scale=1.0,
                )
            else:
                relu_eng.tensor_scalar(
                    out=dst8[:, qs], in0=pp[0:H, :],
                    scalar1=bi128[0:H, bcol : bcol + 1], scalar2=0.0,
                    op0=Add, op1=Max,
                )

        # m pipeline: cast chunk -> transposes -> projection quarter
        for c4 in range(0, NB, 4):
            nc.vector.tensor_copy(
                out=m16[:, c4 : c4 + 4, :], in_=m_nat[:, c4 : c4 + 4, :]
            )
            tp_group(m16, c4, mT16, mT16b, nc.vector if c4 % 8 == 0 else nc.scalar)
            proj(wm16, wm16b, 1, mT16, mT16b, mpT16, c4 // 4, nc.scalar)
        # x: cast + transpose all blocks, project only quarter 0 up front;
        # quarters 1-3 are interleaved into attention h0's PE slack.
        for c4 in range(0, NB, 4):
            nc.vector.tensor_copy(
                out=x16[:, c4 : c4 + 4, :], in_=x_nat[:, c4 : c4 + 4, :]
            )
            tp_group(x16, c4, xT16, xT16b, nc.scalar if c4 % 8 == 0 else nc.vector)
        proj(wi16, wi16b, 0, xT16, xT16b, xpT16, 0, nc.vector)
        proj(wi16, wi16b, 0, xT16, xT16b, xpT16, 1, nc.vector)

        # gate weights fp16 (4 g-chunks)
        wg16 = []
        for gi in range(4):
            w = const.tile(
                [wgst[gi].shape[0], G], F16, tag=f"wg16_{gi}", name=f"wg16_{gi}"
            )
            (nc.gpsimd if gi % 2 == 0 else nc.vector).tensor_copy(
                out=w, in_=wgst[gi]
            )
            wg16.append(w)
        bg_half = []
        for gi in range(3):
            t = const.tile(
                [bg_st[gi].shape[0], 1], F32, tag=f"bgh_{gi}", name=f"bgh_{gi}"
            )
            nc.gpsimd.tensor_scalar_mul(out=t, in0=bg_st[gi], scalar1=0.5)
            bg_half.append(t)

        # ---- attention h0 / h1 -------------------------------------------
        # Per half: 16 j-tiles of scores+exp, 8 DoubleRow U pairs.  The exp
        # (ACT) is the rate limiter, so leftover x projections are slotted
        # into h0's PE stream.  U tiles are copied to SBUF right after each
        # half so the single-buffer PSUM pools can be reused.
        U1c, U2c, rr16n = [], [], {}

        def emit_scores(h, j):
            sp = psb.tile([128, HALF], F32, tag="big", name="sp")
            for s in range(2):
                ss = slice(h * HALF + s * 512, h * HALF + (s + 1) * 512)
                nc.tensor.matmul(
                    sp[:, s * 512 : (s + 1) * 512],
                    mpT16[:, j * 128 : (j + 1) * 128], xpT16[:, ss],
                    start=True, stop=True, skip_group_check=True,
                )
            return sp

        def emit_exp(sp, j, e8, slot):
            nc.scalar.activation(
                out=e8[:, :, slot, :], in_=sp, func=Exp,
                bias=maskf[:, j : j + 1], scale=SCALE,
            )

        def emit_U(U1, U2, e8, jp):
            first, last = jp == 0, jp == 7
            for s in range(2):
                ps = slice(s * 512, (s + 1) * 512)
                nc.tensor.matmul(
                    U1[:, ps], m8a[:, 2 * jp : 2 * jp + 2], e8[:, s],
                    start=first, stop=last, perf_mode=DR, skip_group_check=True,
                )
                nc.tensor.matmul(
                    U2[:, ps], m8b[:, 2 * jp : 2 * jp + 2], e8[:, s],
                    start=first, stop=last, perf_mode=DR, skip_group_check=True,
                )

        def emit_denomT(h):
            # denom row [1,1024] -> [128, 8] via PE transposes (so the DVE
            # reciprocal runs partition-parallel instead of 6.5ns/elem on a
            # single lane), then PE-transpose back to a [1, 1024] row.
            # The [1,1] identity sits at partition 32 to match the denom
            # row's base partition (engine APs need 32-aligned bases).
            dps = psb.tile([128, 8], F32, tag="big", name="dps")
            for c in range(8):
                nc.tensor.transpose(
                    dps[:, c : c + 1],
                    U2c[h][32:33, c * 128 : (c + 1) * 128],
                    ident32[32:33, 32:33],
                )
            rrT = work.tile([128, 8], F32, tag="rrT")
            nc.vector.reciprocal(out=rrT, in_=dps)
            rrT16 = work.tile([128, 8], F16, tag="rrT16")
            nc.vector.tensor_copy(out=rrT16, in_=rrT)
            rrps = psb.tile([1, HALF], F16, tag="big", name="rrps")
            for c in range(8):
                nc.tensor.transpose(
                    rrps[:, c * 128 : (c + 1) * 128],
                    rrT16[:, c : c + 1],
                    ident16,
                )
            rr = work.tile([1, HALF], F16, tag="rr16")
            nc.vector.tensor_copy(out=rr[:, 0:512], in_=rrps[:, 0:512])
            nc.scalar.copy(out=rr[:, 512:HALF], in_=rrps[:, 512:HALF])
            rr16n[h] = rr

        # tail stage emitters (interleaved into the next half's attention
        # stream so the PE queue never drains -- a drained PE re-throttles
        # the HAM clock gate to 1.2 GHz)
        o_re = o_d.rearrange("(q n) k -> q n k", n=NB)
        kranges = [(0, 128), (128, 256), (256, G)]
        tails = {}

        def t_bc(h):
            hs = slice(h * HALF, (h + 1) * HALF)
            st = {}
            bc = psb.tile([128, HALF], F32, tag="big", name="bc")
            for s2 in range(2):
                ps = slice(s2 * 512, (s2 + 1) * 512)
                nc.tensor.matmul(
                    bc[:, ps], ones16, rr16n[h][:, ps],
                    start=True, stop=True, skip_group_check=True,
                )
            RC16 = work.tile([128, HALF], F16, tag="RC16")
            nc.vector.tensor_mul(out=RC16, in0=U1c[h], in1=bc)
            RD16 = work.tile([D - 128, HALF], F16, tag="RD16")
            nc.vector.tensor_mul(
                out=RD16, in0=U2c[h][0 : D - 128, :], in1=bc[0 : D - 128, :]
            )
            # output-aligned res staging (partition shifts via DMA)
            res_mid = work.tile([128, HALF], F16, tag="res_mid")
            nc.vector.tensor_copy(out=res_mid[0 : D - 128, :], in_=xT16b[:, hs])
            nc.sync.dma_start(
                out=res_mid[D - 128 : 128, :], in_=RC16[0 : 256 - D, :]
            )
            res_hi = work.tile([G - 256, HALF], F16, tag="res_hi")
            nc.scalar.dma_start(
                out=res_hi[0 : D - 128, :], in_=RC16[256 - D : 128, :]
            )
            nc.sync.dma_start(out=res_hi[D - 128 : G - 256, :], in_=RD16)
            st["res_g"] = [xT16[:, hs], xT16b[:, hs], RC16, RD16]
            st["res_k"] = [xT16[:, hs], res_mid, res_hi]
            st["oT"] = [
                work.tile([128, HALF], F16, tag="oT0", name="oT0"),
                work.tile([128, HALF], F16, tag="oT1", name="oT1"),
                work.tile([G - 256, HALF], F16, tag="oT2", name="oT2"),
            ]
            st["gp"] = {}
            tails[h] = st

        def t_gate_mm(h, kc, s2):
            st = tails[h]
            k0, k1 = kranges[kc]
            if s2 == 0:
                st["gp"][kc] = psb.tile(
                    [k1 - k0, HALF], F32, tag="big", name="gp"
                )
            gp = st["gp"][kc]
            ps = slice(s2 * 512, (s2 + 1) * 512)
            for gc in range(4):
                nc.tensor.matmul(
                    gp[:, ps], wg16[gc][:, k0:k1], st["res_g"][gc][:, ps],
                    start=(gc == 0), stop=(gc == 3), skip_group_check=True,
                )

        def t_gate_act(h, kc):
            # gate = sigmoid(z) = 0.5*(1+tanh(z/2)); tanh shares the exp ACT
            # table.  (gs+1) folds into the DVE multiply, 0.5 into the final
            # PSUM->SBUF copy.
            st = tails[h]
            k0, k1 = kranges[kc]
            gs = work.tile([k1 - k0, HALF], F16, tag="gs", bufs=3)
            nc.scalar.activation(
                out=gs, in_=st["gp"][kc], func=Tanh, bias=bg_half[kc], scale=0.5
            )
            nc.vector.scalar_tensor_tensor(
                out=st["oT"][kc], in0=gs, scalar=1.0, in1=st["res_k"][kc],
                op0=Add, op1=Mult,
            )

        def t_out(h):
            # transpose back (fp16), scale by 0.5 on the upconverting copy,
            # store 4 blocks per DMA
            oT = tails[h]["oT"]
            for g4 in range(2):
                onat = work.tile([128, 4, 304], F32, tag="onat", bufs=2)
                for i in range(2):
                    op2 = psb.tile([128, 2, 304], F16, tag="big", name="op2")
                    for k in range(2):
                        n_loc = g4 * 4 + i * 2 + k
                        sl = slice(n_loc * 128, (n_loc + 1) * 128)
                        nc.tensor.transpose(
                            op2[:, k, 0:128], oT[0][:, sl], ident16
                        )
                        nc.tensor.transpose(
                            op2[:, k, 128:256], oT[1][:, sl], ident16
                        )
                        nc.tensor.transpose(
                            op2[:, k, 256:G], oT[2][:, sl],
                            ident16[: G - 256, : G - 256],
                        )
                    osl = slice(i * 2, (i + 1) * 2)
                    if i == 0:
                        nc.vector.tensor_scalar_mul(
                            out=onat[:, osl, :], in0=op2, scalar1=0.5
                        )
                    else:
                        nc.scalar.activation(
                            out=onat[:, osl, :], in_=op2, func=Copy,
                            bias=0.0, scale=0.5,
                        )
                    n0 = h * 8 + g4 * 4 + i * 2
                    dq = nc.sync if (g4 * 2 + i) % 2 == 0 else nc.scalar
                    dq.dma_start(
                        out=o_re[:, n0 : n0 + 2, :],
                        in_=onat[:, osl, 0:G],
                    )

        for h in range(2):
            U1 = psu.tile([128, HALF], F32, tag="u1")
            U2 = psu.tile([64, HALF], F32, tag="u2")
            sps, e8s = {}, {}
            e8s[0] = epool.tile([128, 2, 2, 512], F8, tag="e8", name="e8")
            sps[0] = emit_scores(h, 0)
            if dbg and h == 0:
                sp0dbg = const.tile([128, HALF], F32)
                nc.vector.tensor_copy(out=sp0dbg, in_=sps[0])
            sps[1] = emit_scores(h, 1)
            for jp in range(8):
                emit_exp(sps.pop(2 * jp), 2 * jp, e8s[jp], 0)
                if 2 * jp + 2 < 16:
                    sps[2 * jp + 2] = emit_scores(h, 2 * jp + 2)
                emit_exp(sps.pop(2 * jp + 1), 2 * jp + 1, e8s[jp], 1)
                if 2 * jp + 3 < 16:
                    sps[2 * jp + 3] = emit_scores(h, 2 * jp + 3)
                if jp + 1 < 8:
                    e8s[jp + 1] = epool.tile([128, 2, 2, 512], F8, tag="e8", name="e8")
                e8cur = e8s.pop(jp)
                if dbg and h == 0 and jp == 0:
                    e8dbg = const.tile([128, 2, 2, 512], F32)
                    nc.vector.tensor_copy(out=e8dbg, in_=e8cur)
                emit_U(U1, U2, e8cur, jp)
                # interleaved fill work: h0 gets the remaining x projections,
                # h1 gets h0's entire gating tail
                if h == 0 and jp in (1, 3):
                    proj(
                        wi16, wi16b, 0, xT16, xT16b, xpT16,
                        2 + (jp - 1) // 2, nc.vector,
                    )
                if h == 1:
                    if jp == 0:
                        emit_denomT(0)
                    elif jp == 1:
                        t_bc(0)
                    elif jp >= 2:
                        kc, s2 = (jp - 2) // 2, jp % 2
                        t_gate_mm(0, kc, s2)
                        if s2 == 1:
                            t_gate_act(0, kc)
            u1c = work.tile([128, HALF], F32, tag="u1c")
            nc.vector.tensor_copy(out=u1c, in_=U1)
            u2c = work.tile([33, HALF], F32, tag="u2c")
            nc.scalar.copy(out=u2c, in_=U2[0:33, :])
            U1c.append(u1c)
            U2c.append(u2c)

        # ---- trailing tail: h0 stores, then all of h1's tail --------------
        emit_denomT(1)
        t_out(0)
        t_bc(1)
        for kc in range(3):
            t_gate_mm(1, kc, 0)
            t_gate_mm(1, kc, 1)
            t_gate_act(1, kc)
        t_out(1)

        if dbg:
            nc.sync.dma_start(out=dbg["u1"][:], in_=U1c[0])
            nc.sync.dma_start(out=dbg["u2"][:], in_=U2c[0])
            rrf = const.tile([1, HALF], F32)
            nc.vector.tensor_copy(out=rrf, in_=rr16n[0])
            nc.sync.dma_start(out=dbg["rr"][:], in_=rrf)
            nc.sync.dma_start(out=dbg["sp0"][:], in_=sp0dbg)
            nc.sync.dma_start(out=dbg["e8"][:], in_=e8dbg)
            m8af = const.tile([128, NB, 128], F32)
            nc.vector.tensor_copy(out=m8af, in_=m8a)
            nc.sync.dma_start(out=dbg["m8a"][:], in_=m8af)
            nc.sync.dma_start(out=dbg["maskf"][:], in_=maskf)


_NC_CACHE = None


def _build_nc():
    global _NC_CACHE
    if _NC_CACHE is not None:
        return _NC_CACHE
    nc = bacc.Bacc(None, target_bir_lowering=False, debug=False)
    x_d = nc.dram_tensor("x", [JX, D], F32, kind="ExternalInput")
    m_d = nc.dram_tensor("m", [JM, D], F32, kind="ExternalInput")
    mask_d = nc.dram_tensor("mask", [JM], I32, kind="ExternalInput")
    wi_d = nc.dram_tensor("Wi", [D, H], F32, kind="ExternalInput")
    bi_d = nc.dram_tensor("bi", [H], F32, kind="ExternalInput")
    wm_d = nc.dram_tensor("Wm", [D, H], F32, kind="ExternalInput")
    bm_d = nc.dram_tensor("bm", [H], F32, kind="ExternalInput")
    wg_d = nc.dram_tensor("Wg", [G, G], F32, kind="ExternalInput")
    bg_d = nc.dram_tensor("bg", [G], F32, kind="ExternalInput")
    o_d = nc.dram_tensor("out", [JX, G], F32, kind="ExternalOutput")
    dbg = None
    if DEBUG_DUMP:
        dbg = {
            "u1": nc.dram_tensor("dbg_u1", [128, HALF], F32, kind="ExternalOutput"),
            "u2": nc.dram_tensor("dbg_u2", [33, HALF], F32, kind="ExternalOutput"),
            "rr": nc.dram_tensor("dbg_rr", [1, HALF], F32, kind="ExternalOutput"),
            "sp0": nc.dram_tensor("dbg_sp0", [128, HALF], F32, kind="ExternalOutput"),
            "e8": nc.dram_tensor("dbg_e8", [128, 2, 2, 512], F32, kind="ExternalOutput"),
            "m8a": nc.dram_tensor("dbg_m8a", [128, NB, 128], F32, kind="ExternalOutput"),
            "maskf": nc.dram_tensor("dbg_maskf", [128, NB], F32, kind="ExternalOutput"),
        }
    with tile.TileContext(nc) as tc:
        _body(tc, x_d, m_d, mask_d, wi_d, bi_d, wm_d, bm_d, wg_d, bg_d, o_d,
              dbg=dbg)
    nc.finalize()
    _NC_CACHE = nc
    return nc


def _in_maps(inputs, memory, mask, Wi, bi, Wm, bm, Wg, bg):
    maps = []
    for b in range(B):
        maps.append(
            {
                "x": np.ascontiguousarray(inputs[b], dtype=np.float32),
                "m": np.ascontiguousarray(memory[b], dtype=np.float32),
                "mask": np.ascontiguousarray(mask[b], dtype=np.int32),
                "Wi": np.ascontiguousarray(Wi, dtype=np.float32),
                "bi": np.ascontiguousarray(bi, dtype=np.float32),
                "Wm": np.ascontiguousarray(Wm, dtype=np.float32),
                "bm": np.ascontiguousarray(bm, dtype=np.float32),
                "Wg": np.ascontiguousarray(Wg, dtype=np.float32),
                "bg": np.ascontiguousarray(bg, dtype=np.float32),
            }
        )
    return maps


def run_spmd(inputs, memory, mask, Wi, bi, Wm, bm, Wg, bg, **spmd_kwargs):
    """Run the kernel across 8 cores; returns the BassKernelResults."""
    nc = _build_nc()
    maps = _in_maps(
        np.asarray(inputs), np.asarray(memory), np.asarray(mask),
        np.asarray(Wi), np.asarray(bi), np.asarray(Wm), np.asarray(bm),
        np.asarray(Wg), np.asarray(bg),
    )
    return run_bass_kernel_spmd(nc, maps, list(range(B)), **spmd_kwargs)


def kernel(inputs, memory, mask, Wi, bi, Wm, bm, Wg, bg):
    res = run_spmd(inputs, memory, mask, Wi, bi, Wm, bm, Wg, bg)
    out = np.stack([res.results[b]["out"] for b in range(B)], axis=0)
    return out.astype(np.float32)


# revision 21
# speedup vs baseline: 1.3315x; 1.0170x over previous
"""Trainium2 Bass kernel for nn_DotAttention (B=8 data-parallel over 8 cores).

Per core (one batch element), using a jm/jx "permuted block layout":
column c = n*128 + q of any T-layout tile corresponds to row 16q + n of the
natural tensor (from the contiguous "(p n) d" DMA).  The permutation is
consistent across scores / exp / U / gating and undone by the output store
pattern, so it never needs an explicit fixup.

  xp = relu(x @ Wi + bi)          [2048, 96]   (fp16 matmul, DVE relu -> fp8)
  mp = relu(m @ Wm + bm)          [2048, 96]   (fp16 matmul, ACT relu -> fp8)
  S.T[jm, jx] = mp . xp           fp8 DoubleRow matmul (K = 2 x 64)
  E = exp(S.T/sqrt(96) + maskbias)  ACT, fp8 out, two j-tiles per e8 pair
  U.T[d, jx] = mtilde.T @ E       fp8 DoubleRow matmul (2 jm-tiles per inst;
                                   mtilde = [m | pad | 1], so U2 row 32 is the
                                   softmax denominator)
  attn = U / denom                denom reciprocal via PE-transpose to [128,8]
                                   (partition-parallel DVE reciprocal), PE ones
                                   broadcast back to [128, jx]
  out = sigmoid(Wg.T @ res + bg) * res
      = 0.5*(1 + tanh(z/2)) * res   tanh lives in the same ACT table set as
                                   exp, so there are no table reloads; the
                                   (gs+1)*res runs on DVE and the 0.5 folds
                                   into the final PSUM->SBUF copy.

Matmul operands are fp16 except scores/U (fp8e4 DoubleRow); PSUM accumulation
is fp32 everywhere.  The whole tail runs in fp16 (5e-4 relative error budget,
gate threshold is 2e-2).
"""

import math
import os

import numpy as np

import concourse.bass as bass
import concourse.mybir as mybir
import concourse.tile as tile
from concourse import bacc
from concourse.bass_utils import run_bass_kernel_spmd
from concourse.masks import make_identity

F32 = mybir.dt.float32
F16 = mybir.dt.float16
F8 = mybir.dt.float8e4
I32 = mybir.dt.int32
DR = mybir.MatmulPerfMode.DoubleRow

B = 8
JX = 2048
JM = 2048
D = 150
H = 96
G = 300
NB = 16  # 128-column blocks per 2048
HALF = 1024
SCALE = 1.0 / math.sqrt(float(H))
NEG_BIG = 1.0e30


DEBUG_DUMP = bool(os.environ.get("KDBG"))


def _body(tc, x_d, m_d, mask_d, wi_d, bi_d, wm_d, bm_d, wg_d, bg_d, o_d,
          dbg=None):
    nc = tc.nc
    Relu = mybir.ActivationFunctionType.Relu
    Exp = mybir.ActivationFunctionType.Exp
    Tanh = mybir.ActivationFunctionType.Tanh
    Copy = mybir.ActivationFunctionType.Copy
    Add = mybir.AluOpType.add
    Max = mybir.AluOpType.max
    Mult = mybir.AluOpType.mult
    Sub = mybir.AluOpType.subtract

    import contextlib

    with contextlib.ExitStack() as ctx:
        const = ctx.enter_context(tc.tile_pool(name="const", bufs=1))
        work = ctx.enter_context(tc.tile_pool(name="work", bufs=2))
        epool = ctx.enter_context(tc.tile_pool(name="epool", bufs=3))
        psb = ctx.enter_context(tc.tile_pool(name="psb", bufs=2, space="PSUM"))
        psu = ctx.enter_context(tc.tile_pool(name="psu", bufs=1, space="PSUM"))

        # ---- input DMAs first: they are the longest poles ----------------
        # x/m in "(p n) d" layout: 9600B contiguous per partition.
        x_nat = const.tile([128, NB, D], F32)
        m_nat = const.tile([128, NB, D], F32)
        x_re = x_d.rearrange("(p n) d -> p n d", n=NB)
        m_re = m_d.rearrange("(p n) d -> p n d", n=NB)
        mask_sb = const.tile([128, NB], I32)
        wstage = const.tile([128, 2 * H], F32)
        wstage2 = const.tile([D - 128, 2 * H], F32)
        bstage = const.tile([H, 2], F32)
        # m chunks on the scalar queue; weight staging FIRST on sync (x is
        # not needed until after the m transposes), so wi16/wm16 are ready
        # before the m projections want them
        for c in range(4):
            cs = slice(c * 4, (c + 1) * 4)
            nc.scalar.dma_start(out=m_nat[:, cs, :], in_=m_re[:, cs, :])
        nc.sync.dma_start(out=wstage[:, 0:H], in_=wi_d[0:128, :])
        nc.sync.dma_start(out=wstage[:, H : 2 * H], in_=wm_d[0:128, :])
        nc.sync.dma_start(out=wstage2[:, 0:H], in_=wi_d[128:D, :])
        nc.sync.dma_start(out=wstage2[:, H : 2 * H], in_=wm_d[128:D, :])
        nc.sync.dma_start(
            out=bstage[:, 0:1], in_=bi_d.rearrange("(n one) -> n one", one=1)
        )
        nc.sync.dma_start(
            out=bstage[:, 1:2], in_=bm_d.rearrange("(n one) -> n one", one=1)
        )
        for c in range(4):
            cs = slice(c * 4, (c + 1) * 4)
            nc.sync.dma_start(out=x_nat[:, cs, :], in_=x_re[:, cs, :])

        # ---- constants (emitted before the gpsimd DMA dispatches so the
        # identity build isn't stuck behind ~10 serialized 0.7us dispatches)
        ident16 = const.tile([128, 128], F16)
        make_identity(nc, ident16)
        ident32 = const.tile([128, 128], F32)
        make_identity(nc, ident32)
        ones16 = const.tile([1, 128], F16)
        nc.gpsimd.memset(ones16, 1.0)
        # preload the exp ACT table while DMAs land (tanh/relu/copy share it)
        dummy = const.tile([1, 1], F32)
        nc.scalar.activation(
            out=dummy, in_=ident32[0:1, 0:1], func=Exp, scale=1.0
        )

        nc.gpsimd.dma_start(
            out=mask_sb, in_=mask_d.rearrange("(p n) -> p n", n=NB)
        )
        # Wg / bg are needed only ~40us in; queue them behind the inputs
        wgst = []
        for gi, (g0, g1) in enumerate([(0, 128), (128, D), (D, D + 128), (D + 128, G)]):
            wst = const.tile([g1 - g0, G], F32, tag=f"wgst_{gi}", name=f"wgst_{gi}")
            (nc.sync if gi % 2 == 0 else nc.scalar).dma_start(
                out=wst, in_=wg_d[g0:g1, :]
            )
            wgst.append(wst)
        bg_st = []
        for gi, (g0, g1) in enumerate([(0, 128), (128, 256), (256, G)]):
            t = const.tile([g1 - g0, 1], F32, tag=f"bgst_{gi}", name=f"bgst_{gi}")
            nc.scalar.dma_start(
                out=t, in_=bg_d[g0:g1].rearrange("(n one) -> n one", one=1)
            )
            bg_st.append(t)

        # ---- weight prep -------------------------------------------------
        wi16 = const.tile([128, 128], F16)  # cols 96:128 zero-padded
        wm16 = const.tile([128, 128], F16)
        nc.gpsimd.memset(wi16[:, H:128], 0.0)
        nc.gpsimd.memset(wm16[:, H:128], 0.0)
        nc.gpsimd.tensor_copy(out=wi16[:, 0:H], in_=wstage[:, 0:H])
        nc.gpsimd.tensor_copy(out=wm16[:, 0:H], in_=wstage[:, H : 2 * H])
        wi16b = const.tile([D - 128, 128], F16)
        wm16b = const.tile([D - 128, 128], F16)
        nc.gpsimd.memset(wi16b[:, H:128], 0.0)
        nc.gpsimd.memset(wm16b[:, H:128], 0.0)
        nc.gpsimd.tensor_copy(out=wi16b[:, 0:H], in_=wstage2[:, 0:H])
        nc.gpsimd.tensor_copy(out=wm16b[:, 0:H], in_=wstage2[:, H : 2 * H])
        bi128 = const.tile([128, 2], F32)  # col 0 = bi, col 1 = bm; rows 96+ = 0
        nc.gpsimd.memset(bi128[H:128, :], 0.0)
        nc.gpsimd.tensor_copy(out=bi128[0:H, :], in_=bstage)

        # mask -> additive exp bias (mask-1)*1e30, natural layout (no transpose)
        maskf = const.tile([128, NB], F32)
        nc.gpsimd.tensor_copy(out=maskf, in_=mask_sb)
        nc.gpsimd.tensor_scalar(
            out=maskf, in0=maskf, scalar1=1.0, scalar2=NEG_BIG,
            op0=Sub, op1=Mult,
        )

        # mtilde fp8 U stationaries, (j, d)-contiguous so the DoubleRow
        # weight slices [:, 2jp:2jp+2, :] merge into one run (ISA req):
        # m8a = m cols 0:128; m8b = [m cols 128:150 | 0-pad | 1] (denominator
        # column lands at U2 partition 32).
        m8a = const.tile([128, NB, 128], F8)
        m8b = const.tile([128, NB, 64], F8)
        nc.gpsimd.memset(m8b[:, :, 22:32], 0.0)
        nc.gpsimd.memset(m8b[:, :, 32:33], 1.0)
        nc.gpsimd.memset(m8b[:, :, 33:64], 0.0)

        # ---- fp16 casts + transposes into T layout -----------------------
        m16 = const.tile([128, NB, D], F16)
        x16 = const.tile([128, NB, D], F16)
        mT16 = const.tile([128, JM], F16)
        mT16b = const.tile([D - 128, JM], F16)
        xT16 = const.tile([128, JX], F16)
        xT16b = const.tile([D - 128, JX], F16)
        # projected activations, fp16 T layout (scores run as plain fp16
        # matmuls: DoubleRow streams both k-planes serially, so DR would
        # DOUBLE the score cost vs one K=96 fp16 pass)
        mpT16 = const.tile([96, JM], F16)
        xpT16 = const.tile([96, JX], F16)

        def tp_group(src16, blk4, dT, dTb, eng):
            # 4 block-transposes batched into one PSUM tile + one copy
            sl4 = slice(blk4 * 128, (blk4 + 4) * 128)
            t1 = psb.tile([128, 512], F16, tag="big", name="tp1")
            t2 = psb.tile([D - 128, 512], F16, tag="big", name="tp2")
            for i in range(4):
                ps = slice(i * 128, (i + 1) * 128)
                nc.tensor.transpose(t1[:, ps], src16[:, blk4 + i, 0:128], ident16)
                nc.tensor.transpose(t2[:, ps], src16[:, blk4 + i, 128:D], ident16)
            (eng.tensor_copy if eng is not nc.scalar else eng.copy)(
                out=dT[:, sl4], in_=t1
            )
            (eng.tensor_copy if eng is not nc.scalar else eng.copy)(
                out=dTb[:, sl4], in_=t2
            )

        def proj(wa, wb, bcol, srcT, srcTb, dst8, q, relu_eng):
            qs = slice(q * 512, (q + 1) * 512)
            pp = psb.tile([128, 512], F32, tag="big", name="pp")
            nc.tensor.matmul(
                pp, wa, srcT[:, qs], start=True, stop=False, skip_group_check=True
            )
            nc.tensor.matmul(
                pp, wb, srcTb[:, qs], start=False, stop=True, skip_group_check=True
            )
            if relu_eng is nc.scalar:
                nc.scalar.activation(
                    out=dst8[:, qs], in_=pp[0:H, :], func=Relu,
                    bias=bi128[0:H, bcol : bcol + 1], scale=1.0,
                )
            else:
                relu_eng.tensor_scalar(
                    out=dst8[:, qs], in0=pp[0:H, :],
                    scalar1=bi128[0:H, bcol : bcol + 1], scalar2=0.0,
                    op0=Add, op1=Max,
                )

        # m pipeline: cast chunk -> transposes -> projection quarter
        for c4 in range(0, NB, 4):
            nc.vector.tensor_copy(
                out=m16[:, c4 : c4 + 4, :], in_=m_nat[:, c4 : c4 + 4, :]
            )
            tp_group(m16, c4, mT16, mT16b, nc.vector if c4 % 8 == 0 else nc.scalar)
            proj(wm16, wm16b, 1, mT16, mT16b, mpT16, c4 // 4, nc.scalar)
        # x: cast + transpose all blocks, project only quarter 0 up front;
        # quarters 1-3 are interleaved into attention h0's PE slack.
        for c4 in range(0, NB, 4):
            nc.vector.tensor_copy(
                out=x16[:, c4 : c4 + 4, :], in_=x_nat[:, c4 : c4 + 4, :]
            )
            tp_group(x16, c4, xT16, xT16b, nc.scalar if c4 % 8 == 0 else nc.vector)
        proj(wi16, wi16b, 0, xT16, xT16b, xpT16, 0, nc.vector)
        proj(wi16, wi16b, 0, xT16, xT16b, xpT16, 1, nc.vector)
        # mtilde fp8 casts (needed by the first U pair, ~8us after this)
        for c in range(2):
            cs = slice(c * 8, (c + 1) * 8)
            nc.vector.tensor_copy(out=m8a[:, cs, :], in_=m_nat[:, cs, 0:128])
            nc.vector.tensor_copy(out=m8b[:, cs, 0:22], in_=m_nat[:, cs, 128:D])

        # gate weights fp16 (4 g-chunks)
        wg16 = []
        for gi in range(4):
            w = const.tile(
                [wgst[gi].shape[0], G], F16, tag=f"wg16_{gi}", name=f"wg16_{gi}"
            )
            (nc.gpsimd if gi % 2 == 0 else nc.vector).tensor_copy(
                out=w, in_=wgst[gi]
            )
            wg16.append(w)
        bg_half = []
        for gi in range(3):
            t = const.tile(
                [bg_st[gi].shape[0], 1], F32, tag=f"bgh_{gi}", name=f"bgh_{gi}"
            )
            nc.gpsimd.tensor_scalar_mul(out=t, in0=bg_st[gi], scalar1=0.5)
            bg_half.append(t)

        # ---- attention h0 / h1 -------------------------------------------
        # Per half: 16 j-tiles of scores+exp, 8 DoubleRow U pairs.  The exp
        # (ACT) is the rate limiter, so leftover x projections are slotted
        # into h0's PE stream.  U tiles are copied to SBUF right after each
        # half so the single-buffer PSUM pools can be reused.
        U1c, U2c, rr16n = [], [], {}

        def emit_scores(h, j):
            sp = psb.tile([128, HALF], F32, tag="big", name="sp")
            for s in range(2):
                ss = slice(h * HALF + s * 512, h * HALF + (s + 1) * 512)
                nc.tensor.matmul(
                    sp[:, s * 512 : (s + 1) * 512],
                    mpT16[:, j * 128 : (j + 1) * 128], xpT16[:, ss],
                    start=True, stop=True, skip_group_check=True,
                )
            return sp

        def emit_exp(sp, j, e8, slot):
            nc.scalar.activation(
                out=e8[:, :, slot, :], in_=sp, func=Exp,
                bias=maskf[:, j : j + 1], scale=SCALE,
            )

        def emit_U(U1, U2, e8, jp):
            first, last = jp == 0, jp == 7
            for s in range(2):
                ps = slice(s * 512, (s + 1) * 512)
                nc.tensor.matmul(
                    U1[:, ps], m8a[:, 2 * jp : 2 * jp + 2], e8[:, s],
                    start=first, stop=last, perf_mode=DR, skip_group_check=True,
                )
                nc.tensor.matmul(
                    U2[:, ps], m8b[:, 2 * jp : 2 * jp + 2], e8[:, s],
                    start=first, stop=last, perf_mode=DR, skip_group_check=True,
                )

        def emit_denomT(h):
            # denom row [1,1024] -> [128, 8] via PE transposes (so the DVE
            # reciprocal runs partition-parallel instead of 6.5ns/elem on a
            # single lane), then PE-transpose back to a [1, 1024] row.
            # The [1,1] identity sits at partition 32 to match the denom
            # row's base partition (engine APs need 32-aligned bases).
            dps = psb.tile([128, 8], F32, tag="big", name="dps")
            for c in range(8):
                nc.tensor.transpose(
                    dps[:, c : c + 1],
                    U2c[h][32:33, c * 128 : (c + 1) * 128],
                    ident32[32:33, 32:33],
                )
            rrT = work.tile([128, 8], F32, tag="rrT")
            nc.vector.reciprocal(out=rrT, in_=dps)
            rrT16 = work.tile([128, 8], F16, tag="rrT16")
            nc.vector.tensor_copy(out=rrT16, in_=rrT)
            rrps = psb.tile([1, HALF], F16, tag="big", name="rrps")
            for c in range(8):
                nc.tensor.transpose(
                    rrps[:, c * 128 : (c + 1) * 128],
                    rrT16[:, c : c + 1],
                    ident16,
                )
            rr = work.tile([1, HALF], F16, tag="rr16")
            nc.vector.tensor_copy(out=rr[:, 0:512], in_=rrps[:, 0:512])
            nc.scalar.copy(out=rr[:, 512:HALF], in_=rrps[:, 512:HALF])
            rr16n[h] = rr

        # tail stage emitters (interleaved into the next half's attention
        # stream so the PE queue never drains -- a drained PE re-throttles
        # the HAM clock gate to 1.2 GHz)
        o_re = o_d.rearrange("(q n) k -> q n k", n=NB)
        kranges = [(0, 128), (128, 256), (256, G)]
        tails = {}

        def t_bc(h):
            hs = slice(h * HALF, (h + 1) * HALF)
            st = {}
            bc = psb.tile([128, HALF], F32, tag="big", name="bc")
            for s2 in range(2):
                ps = slice(s2 * 512, (s2 + 1) * 512)
                nc.tensor.matmul(
                    bc[:, ps], ones16, rr16n[h][:, ps],
                    start=True, stop=True, skip_group_check=True,
                )
            RC16 = work.tile([128, HALF], F16, tag="RC16")
            nc.vector.tensor_mul(out=RC16, in0=U1c[h], in1=bc)
            RD16 = work.tile([D - 128, HALF], F16, tag="RD16")
            nc.vector.tensor_mul(
                out=RD16, in0=U2c[h][0 : D - 128, :], in1=bc[0 : D - 128, :]
            )
            # output-aligned res staging (partition shifts via DMA)
            res_mid = work.tile([128, HALF], F16, tag="res_mid")
            nc.vector.tensor_copy(out=res_mid[0 : D - 128, :], in_=xT16b[:, hs])
            nc.sync.dma_start(
                out=res_mid[D - 128 : 128, :], in_=RC16[0 : 256 - D, :]
            )
            res_hi = work.tile([G - 256, HALF], F16, tag="res_hi")
            nc.scalar.dma_start(
                out=res_hi[0 : D - 128, :], in_=RC16[256 - D : 128, :]
            )
            nc.sync.dma_start(out=res_hi[D - 128 : G - 256, :], in_=RD16)
            st["res_g"] = [xT16[:, hs], xT16b[:, hs], RC16, RD16]
            st["res_k"] = [xT16[:, hs], res_mid, res_hi]
            st["oT"] = [
                work.tile([128, HALF], F16, tag="oT0", name="oT0"),
                work.tile([128, HALF], F16, tag="oT1", name="oT1"),
                work.tile([G - 256, HALF], F16, tag="oT2", name="oT2"),
            ]
            st["gp"] = {}
            tails[h] = st

        def t_gate_mm(h, kc, s2):
            st = tails[h]
            k0, k1 = kranges[kc]
            if s2 == 0:
                st["gp"][kc] = psb.tile(
                    [k1 - k0, HALF], F32, tag="big", name="gp"
                )
            gp = st["gp"][kc]
            ps = slice(s2 * 512, (s2 + 1) * 512)
            for gc in range(4):
                nc.tensor.matmul(
                    gp[:, ps], wg16[gc][:, k0:k1], st["res_g"][gc][:, ps],
                    start=(gc == 0), stop=(gc == 3), skip_group_check=True,
                )

        def t_gate_act(h, kc):
            # gate = sigmoid(z) = 0.5*(1+tanh(z/2)); tanh shares the exp ACT
            # table.  (gs+1) folds into the DVE multiply, 0.5 into the final
            # PSUM->SBUF copy.
            st = tails[h]
            k0, k1 = kranges[kc]
            gs = work.tile([k1 - k0, HALF], F16, tag="gs", bufs=3)
            nc.scalar.activation(
                out=gs, in_=st["gp"][kc], func=Tanh, bias=bg_half[kc], scale=0.5
            )
            nc.vector.scalar_tensor_tensor(
                out=st["oT"][kc], in0=gs, scalar=1.0, in1=st["res_k"][kc],
                op0=Add, op1=Mult,
            )

        def t_out(h):
            # transpose back (fp16), scale by 0.5 on the upconverting copy,
            # store 4 blocks per DMA
            oT = tails[h]["oT"]
            for g4 in range(2):
                onat = work.tile([128, 4, 304], F32, tag="onat", bufs=2)
                for i in range(2):
                    op2 = psb.tile([128, 2, 304], F16, tag="big", name="op2")
                    for k in range(2):
                        n_loc = g4 * 4 + i * 2 + k
                        sl = slice(n_loc * 128, (n_loc + 1) * 128)
                        nc.tensor.transpose(
                            op2[:, k, 0:128], oT[0][:, sl], ident16
                        )
                        nc.tensor.transpose(
                            op2[:, k, 128:256], oT[1][:, sl], ident16
                        )
                        nc.tensor.transpose(
                            op2[:, k, 256:G], oT[2][:, sl],
                            ident16[: G - 256, : G - 256],
                        )
                    osl = slice(i * 2, (i + 1) * 2)
                    if i == 0:
                        nc.vector.tensor_scalar_mul(
                            out=onat[:, osl, :], in0=op2, scalar1=0.5
                        )
                    else:
                        nc.scalar.activation(
                            out=onat[:, osl, :], in_=op2, func=Copy,
                            bias=0.0, scale=0.5,
                        )
                    n0 = h * 8 + g4 * 4 + i * 2
                    dq = nc.sync if (g4 * 2 + i) % 2 == 0 else nc.scalar
                    dq.dma_start(
                        out=o_re[:, n0 : n0 + 2, :],
                        in_=onat[:, osl, 0:G],
                    )

        for h in range(2):
            U1 = psu.tile([128, HALF], F32, tag="u1")
            U2 = psu.tile([64, HALF], F32, tag="u2")
            sps, e8s = {}, {}
            e8s[0] = epool.tile([128, 2, 2, 512], F8, tag="e8", name="e8")
            sps[0] = emit_scores(h, 0)
            if dbg and h == 0:
                sp0dbg = const.tile([128, HALF], F32)
                nc.vector.tensor_copy(out=sp0dbg, in_=sps[0])
            sps[1] = emit_scores(h, 1)
            for jp in range(8):
                emit_exp(sps.pop(2 * jp), 2 * jp, e8s[jp], 0)
                if 2 * jp + 2 < 16:
                    sps[2 * jp + 2] = emit_scores(h, 2 * jp + 2)
                emit_exp(sps.pop(2 * jp + 1), 2 * jp + 1, e8s[jp], 1)
                if 2 * jp + 3 < 16:
                    sps[2 * jp + 3] = emit_scores(h, 2 * jp + 3)
                if jp + 1 < 8:
                    e8s[jp + 1] = epool.tile([128, 2, 2, 512], F8, tag="e8", name="e8")
                e8cur = e8s.pop(jp)
                if dbg and h == 0 and jp == 0:
                    e8dbg = const.tile([128, 2, 2, 512], F32)
                    nc.vector.tensor_copy(out=e8dbg, in_=e8cur)
                emit_U(U1, U2, e8cur, jp)
                # interleaved fill work: h0 gets the remaining x projections,
                # h1 gets h0's entire gating tail
                if h == 0 and jp in (1, 3):
                    proj(
                        wi16, wi16b, 0, xT16, xT16b, xpT16,
                        2 + (jp - 1) // 2, nc.vector,
                    )
                if h == 1:
                    if jp == 0:
                        emit_denomT(0)
                    elif jp == 1:
                        t_bc(0)
                    elif jp >= 2:
                        kc, s2 = (jp - 2) // 2, jp % 2
                        t_gate_mm(0, kc, s2)
                        if s2 == 1:
                            t_gate_act(0, kc)
            u1c = work.tile([128, HALF], F32, tag="u1c")
            nc.vector.tensor_copy(out=u1c, in_=U1)
            u2c = work.tile([33, HALF], F32, tag="u2c")
            nc.scalar.copy(out=u2c, in_=U2[0:33, :])
            U1c.append(u1c)
            U2c.append(u2c)

        # ---- trailing tail: h0 stores, then all of h1's tail --------------
        emit_denomT(1)
        t_out(0)
        t_bc(1)
        for kc in range(3):
            t_gate_mm(1, kc, 0)
            t_gate_mm(1, kc, 1)
            t_gate_act(1, kc)
        t_out(1)

        if dbg:
            nc.sync.dma_start(out=dbg["u1"][:], in_=U1c[0])
            nc.sync.dma_start(out=dbg["u2"][:], in_=U2c[0])
            rrf = const.tile([1, HALF], F32)
            nc.vector.tensor_copy(out=rrf, in_=rr16n[0])
            nc.sync.dma_start(out=dbg["rr"][:], in_=rrf)
            nc.sync.dma_start(out=dbg["sp0"][:], in_=sp0dbg)
            nc.sync.dma_start(out=dbg["e8"][:], in_=e8dbg)
            m8af = const.tile([128, NB, 128], F32)
            nc.vector.tensor_copy(out=m8af, in_=m8a)
            nc.sync.dma_start(out=dbg["m8a"][:], in_=m8af)
            nc.sync.dma_start(out=dbg["maskf"][:], in_=maskf)


_NC_CACHE = None


def _build_nc():
    global _NC_CACHE
    if _NC_CACHE is not None:
        return _NC_CACHE
    nc = bacc.Bacc(None, target_bir_lowering=False, debug=False)
    x_d = nc.dram_tensor("x", [JX, D], F32, kind="ExternalInput")
    m_d = nc.dram_tensor("m", [JM, D], F32, kind="ExternalInput")
    mask_d = nc.dram_tensor("mask", [JM], I32, kind="ExternalInput")
    wi_d = nc.dram_tensor("Wi", [D, H], F32, kind="ExternalInput")
    bi_d = nc.dram_tensor("bi", [H], F32, kind="ExternalInput")
    wm_d = nc.dram_tensor("Wm", [D, H], F32, kind="ExternalInput")
    bm_d = nc.dram_tensor("bm", [H], F32, kind="ExternalInput")
    wg_d = nc.dram_tensor("Wg", [G, G], F32, kind="ExternalInput")
    bg_d = nc.dram_tensor("bg", [G], F32, kind="ExternalInput")
    o_d = nc.dram_tensor("out", [JX, G], F32, kind="ExternalOutput")
    dbg = None
    if DEBUG_DUMP:
        dbg = {
            "u1": nc.dram_tensor("dbg_u1", [128, HALF], F32, kind="ExternalOutput"),
            "u2": nc.dram_tensor("dbg_u2", [33, HALF], F32, kind="ExternalOutput"),
            "rr": nc.dram_tensor("dbg_rr", [1, HALF], F32, kind="ExternalOutput"),
            "sp0": nc.dram_tensor("dbg_sp0", [128, HALF], F32, kind="ExternalOutput"),
            "e8": nc.dram_tensor("dbg_e8", [128, 2, 2, 512], F32, kind="ExternalOutput"),
            "m8a": nc.dram_tensor("dbg_m8a", [128, NB, 128], F32, kind="ExternalOutput"),
            "maskf": nc.dram_tensor("dbg_maskf", [128, NB], F32, kind="ExternalOutput"),
        }
    with tile.TileContext(nc) as tc:
        _body(tc, x_d, m_d, mask_d, wi_d, bi_d, wm_d, bm_d, wg_d, bg_d, o_d,
              dbg=dbg)
    nc.finalize()
    _NC_CACHE = nc
    return nc


def _in_maps(inputs, memory, mask, Wi, bi, Wm, bm, Wg, bg):
    maps = []
    for b in range(B):
        maps.append(
            {
                "x": np.ascontiguousarray(inputs[b], dtype=np.float32),
                "m": np.ascontiguousarray(memory[b], dtype=np.float32),
                "mask": np.ascontiguousarray(mask[b], dtype=np.int32),
                "Wi": np.ascontiguousarray(Wi, dtype=np.float32),
                "bi": np.ascontiguousarray(bi, dtype=np.float32),
                "Wm": np.ascontiguousarray(Wm, dtype=np.float32),
                "bm": np.ascontiguousarray(bm, dtype=np.float32),
                "Wg": np.ascontiguousarray(Wg, dtype=np.float32),
                "bg": np.ascontiguousarray(bg, dtype=np.float32),
            }
        )
    return maps


def run_spmd(inputs, memory, mask, Wi, bi, Wm, bm, Wg, bg, **spmd_kwargs):
    """Run the kernel across 8 cores; returns the BassKernelResults."""
    nc = _build_nc()
    maps = _in_maps(
        np.asarray(inputs), np.asarray(memory), np.asarray(mask),
        np.asarray(Wi), np.asarray(bi), np.asarray(Wm), np.asarray(bm),
        np.asarray(Wg), np.asarray(bg),
    )
    return run_bass_kernel_spmd(nc, maps, list(range(B)), **spmd_kwargs)


def kernel(inputs, memory, mask, Wi, bi, Wm, bm, Wg, bg):
    res = run_spmd(inputs, memory, mask, Wi, bi, Wm, bm, Wg, bg)
    out = np.stack([res.results[b]["out"] for b in range(B)], axis=0)
    return out.astype(np.float32)


# revision 22
# speedup vs baseline: 1.3323x; 1.0006x over previous
"""Trainium2 Bass kernel for nn_DotAttention (B=8 data-parallel over 8 cores).

Per core (one batch element), using a jm/jx "permuted block layout":
column c = n*128 + q of any T-layout tile corresponds to row 16q + n of the
natural tensor (from the contiguous "(p n) d" DMA).  The permutation is
consistent across scores / exp / U / gating and undone by the output store
pattern, so it never needs an explicit fixup.

  xp = relu(x @ Wi + bi)          [2048, 96]   (fp16 matmul, DVE relu -> fp8)
  mp = relu(m @ Wm + bm)          [2048, 96]   (fp16 matmul, ACT relu -> fp8)
  S.T[jm, jx] = mp . xp           fp8 DoubleRow matmul (K = 2 x 64)
  E = exp(S.T/sqrt(96) + maskbias)  ACT, fp8 out, two j-tiles per e8 pair
  U.T[d, jx] = mtilde.T @ E       fp8 DoubleRow matmul (2 jm-tiles per inst;
                                   mtilde = [m | pad | 1], so U2 row 32 is the
                                   softmax denominator)
  attn = U / denom                denom reciprocal via PE-transpose to [128,8]
                                   (partition-parallel DVE reciprocal), PE ones
                                   broadcast back to [128, jx]
  out = sigmoid(Wg.T @ res + bg) * res
      = 0.5*(1 + tanh(z/2)) * res   tanh lives in the same ACT table set as
                                   exp, so there are no table reloads; the
                                   (gs+1)*res runs on DVE and the 0.5 folds
                                   into the final PSUM->SBUF copy.

Matmul operands are fp16 except scores/U (fp8e4 DoubleRow); PSUM accumulation
is fp32 everywhere.  The whole tail runs in fp16 (5e-4 relative error budget,
gate threshold is 2e-2).
"""

import math
import os

import numpy as np

import concourse.bass as bass
import concourse.mybir as mybir
import concourse.tile as tile
from concourse import bacc
from concourse.bass_utils import run_bass_kernel_spmd
from concourse.masks import make_identity

F32 = mybir.dt.float32
F16 = mybir.dt.float16
F8 = mybir.dt.float8e4
I32 = mybir.dt.int32
DR = mybir.MatmulPerfMode.DoubleRow

B = 8
JX = 2048
JM = 2048
D = 150
H = 96
G = 300
NB = 16  # 128-column blocks per 2048
HALF = 1024
SCALE = 1.0 / math.sqrt(float(H))
NEG_BIG = 1.0e30


DEBUG_DUMP = bool(os.environ.get("KDBG"))


def _body(tc, x_d, m_d, mask_d, wi_d, bi_d, wm_d, bm_d, wg_d, bg_d, o_d,
          dbg=None):
    nc = tc.nc
    Relu = mybir.ActivationFunctionType.Relu
    Exp = mybir.ActivationFunctionType.Exp
    Tanh = mybir.ActivationFunctionType.Tanh
    Copy = mybir.ActivationFunctionType.Copy
    Add = mybir.AluOpType.add
    Max = mybir.AluOpType.max
    Mult = mybir.AluOpType.mult
    Sub = mybir.AluOpType.subtract

    import contextlib

    with contextlib.ExitStack() as ctx:
        const = ctx.enter_context(tc.tile_pool(name="const", bufs=1))
        work = ctx.enter_context(tc.tile_pool(name="work", bufs=2))
        epool = ctx.enter_context(tc.tile_pool(name="epool", bufs=3))
        psb = ctx.enter_context(tc.tile_pool(name="psb", bufs=2, space="PSUM"))
        psu = ctx.enter_context(tc.tile_pool(name="psu", bufs=1, space="PSUM"))

        # ---- input DMAs first: they are the longest poles ----------------
        # x/m in "(p n) d" layout: 9600B contiguous per partition.
        x_nat = const.tile([128, NB, D], F32)
        m_nat = const.tile([128, NB, D], F32)
        x_re = x_d.rearrange("(p n) d -> p n d", n=NB)
        m_re = m_d.rearrange("(p n) d -> p n d", n=NB)
        mask_sb = const.tile([128, NB], I32)
        wstage = const.tile([128, 2 * H], F32)
        wstage2 = const.tile([D - 128, 2 * H], F32)
        bstage = const.tile([H, 2], F32)
        # m chunks on the scalar queue; weight staging FIRST on sync (x is
        # not needed until after the m transposes), so wi16/wm16 are ready
        # before the m projections want them
        for c in range(4):
            cs = slice(c * 4, (c + 1) * 4)
            nc.scalar.dma_start(out=m_nat[:, cs, :], in_=m_re[:, cs, :])
        nc.sync.dma_start(out=wstage[:, 0:H], in_=wi_d[0:128, :])
        nc.sync.dma_start(out=wstage[:, H : 2 * H], in_=wm_d[0:128, :])
        nc.sync.dma_start(out=wstage2[:, 0:H], in_=wi_d[128:D, :])
        nc.sync.dma_start(out=wstage2[:, H : 2 * H], in_=wm_d[128:D, :])
        nc.sync.dma_start(
            out=bstage[:, 0:1], in_=bi_d.rearrange("(n one) -> n one", one=1)
        )
        nc.sync.dma_start(
            out=bstage[:, 1:2], in_=bm_d.rearrange("(n one) -> n one", one=1)
        )
        for c in range(4):
            cs = slice(c * 4, (c + 1) * 4)
            nc.sync.dma_start(out=x_nat[:, cs, :], in_=x_re[:, cs, :])

        # ---- constants (emitted before the gpsimd DMA dispatches so the
        # identity build isn't stuck behind ~10 serialized 0.7us dispatches)
        ident16 = const.tile([128, 128], F16)
        make_identity(nc, ident16)
        ident32 = const.tile([128, 128], F32)
        make_identity(nc, ident32)
        ones16 = const.tile([1, 128], F16)
        nc.gpsimd.memset(ones16, 1.0)
        # preload the exp ACT table while DMAs land (tanh/relu/copy share it)
        dummy = const.tile([1, 1], F32)
        nc.scalar.activation(
            out=dummy, in_=ident32[0:1, 0:1], func=Exp, scale=1.0
        )

        nc.gpsimd.dma_start(
            out=mask_sb, in_=mask_d.rearrange("(p n) -> p n", n=NB)
        )
        # Wg / bg are needed only ~40us in; queue them behind the inputs
        wgst = []
        for gi, (g0, g1) in enumerate([(0, 128), (128, D), (D, D + 128), (D + 128, G)]):
            wst = const.tile([g1 - g0, G], F32, tag=f"wgst_{gi}", name=f"wgst_{gi}")
            (nc.sync if gi % 2 == 0 else nc.scalar).dma_start(
                out=wst, in_=wg_d[g0:g1, :]
            )
            wgst.append(wst)
        bg_st = []
        for gi, (g0, g1) in enumerate([(0, 128), (128, 256), (256, G)]):
            t = const.tile([g1 - g0, 1], F32, tag=f"bgst_{gi}", name=f"bgst_{gi}")
            nc.scalar.dma_start(
                out=t, in_=bg_d[g0:g1].rearrange("(n one) -> n one", one=1)
            )
            bg_st.append(t)

        # ---- weight prep -------------------------------------------------
        wi16 = const.tile([128, 128], F16)  # cols 96:128 zero-padded
        wm16 = const.tile([128, 128], F16)
        nc.gpsimd.memset(wi16[:, H:128], 0.0)
        nc.gpsimd.memset(wm16[:, H:128], 0.0)
        nc.gpsimd.tensor_copy(out=wi16[:, 0:H], in_=wstage[:, 0:H])
        nc.gpsimd.tensor_copy(out=wm16[:, 0:H], in_=wstage[:, H : 2 * H])
        wi16b = const.tile([D - 128, 128], F16)
        wm16b = const.tile([D - 128, 128], F16)
        nc.gpsimd.memset(wi16b[:, H:128], 0.0)
        nc.gpsimd.memset(wm16b[:, H:128], 0.0)
        nc.gpsimd.tensor_copy(out=wi16b[:, 0:H], in_=wstage2[:, 0:H])
        nc.gpsimd.tensor_copy(out=wm16b[:, 0:H], in_=wstage2[:, H : 2 * H])
        bi128 = const.tile([128, 2], F32)  # col 0 = bi, col 1 = bm; rows 96+ = 0
        nc.gpsimd.memset(bi128[H:128, :], 0.0)
        nc.gpsimd.tensor_copy(out=bi128[0:H, :], in_=bstage)

        # mask -> additive exp bias (mask-1)*1e30, natural layout (no transpose)
        maskf = const.tile([128, NB], F32)
        nc.gpsimd.tensor_copy(out=maskf, in_=mask_sb)
        nc.gpsimd.tensor_scalar(
            out=maskf, in0=maskf, scalar1=1.0, scalar2=NEG_BIG,
            op0=Sub, op1=Mult,
        )

        # mtilde fp8 U stationaries, (j, d)-contiguous so the DoubleRow
        # weight slices [:, 2jp:2jp+2, :] merge into one run (ISA req):
        # m8a = m cols 0:128; m8b = [m cols 128:150 | 0-pad | 1] (denominator
        # column lands at U2 partition 32).
        m8a = const.tile([128, NB, 128], F8)
        m8b = const.tile([128, NB, 64], F8)
        nc.gpsimd.memset(m8b[:, :, 22:32], 0.0)
        nc.gpsimd.memset(m8b[:, :, 32:33], 1.0)
        nc.gpsimd.memset(m8b[:, :, 33:64], 0.0)

        # ---- fp16 casts + transposes into T layout -----------------------
        m16 = const.tile([128, NB, D], F16)
        x16 = const.tile([128, NB, D], F16)
        mT16 = const.tile([128, JM], F16)
        mT16b = const.tile([D - 128, JM], F16)
        xT16 = const.tile([128, JX], F16)
        xT16b = const.tile([D - 128, JX], F16)
        # projected activations, fp8 T layout.  Scores run as plain fp8
        # matmuls (NO DoubleRow: DR streams both k-planes serially, doubling
        # cost vs one K=96 pass; plain fp8 matches bf16 speed but draws much
        # less PE power, which keeps the HAM power throttle away).
        mpT16 = const.tile([96, JM], F8)
        xpT16 = const.tile([96, JX], F8)

        def tp_group(src16, blk4, dT, dTb, eng):
            # 4 block-transposes batched into one PSUM tile + one copy
            sl4 = slice(blk4 * 128, (blk4 + 4) * 128)
            t1 = psb.tile([128, 512], F16, tag="big", name="tp1")
            t2 = psb.tile([D - 128, 512], F16, tag="big", name="tp2")
            for i in range(4):
                ps = slice(i * 128, (i + 1) * 128)
                nc.tensor.transpose(t1[:, ps], src16[:, blk4 + i, 0:128], ident16)
                nc.tensor.transpose(t2[:, ps], src16[:, blk4 + i, 128:D], ident16)
            (eng.tensor_copy if eng is not nc.scalar else eng.copy)(
                out=dT[:, sl4], in_=t1
            )
            (eng.tensor_copy if eng is not nc.scalar else eng.copy)(
                out=dTb[:, sl4], in_=t2
            )

        def proj(wa, wb, bcol, srcT, srcTb, dst8, q, relu_eng):
            qs = slice(q * 512, (q + 1) * 512)
            pp = psb.tile([128, 512], F32, tag="big", name="pp")
            nc.tensor.matmul(
                pp, wa, srcT[:, qs], start=True, stop=False, skip_group_check=True
            )
            nc.tensor.matmul(
                pp, wb, srcTb[:, qs], start=False, stop=True, skip_group_check=True
            )
            if relu_eng is nc.scalar:
                nc.scalar.activation(
                    out=dst8[:, qs], in_=pp[0:H, :], func=Relu,
                    bias=bi128[0:H, bcol : bcol + 1], scale=1.0,
                )
            else:
                relu_eng.tensor_scalar(
                    out=dst8[:, qs], in0=pp[0:H, :],
                    scalar1=bi128[0:H, bcol : bcol + 1], scalar2=0.0,
                    op0=Add, op1=Max,
                )

        # m pipeline: cast chunk -> transposes -> projection quarter
        for c4 in range(0, NB, 4):
            nc.vector.tensor_copy(
                out=m16[:, c4 : c4 + 4, :], in_=m_nat[:, c4 : c4 + 4, :]
            )
            tp_group(m16, c4, mT16, mT16b, nc.vector if c4 % 8 == 0 else nc.scalar)
            proj(wm16, wm16b, 1, mT16, mT16b, mpT16, c4 // 4, nc.scalar)
        # x: cast + transpose all blocks, project only quarter 0 up front;
        # quarters 1-3 are interleaved into attention h0's PE slack.
        for c4 in range(0, NB, 4):
            nc.vector.tensor_copy(
                out=x16[:, c4 : c4 + 4, :], in_=x_nat[:, c4 : c4 + 4, :]
            )
            tp_group(x16, c4, xT16, xT16b, nc.scalar if c4 % 8 == 0 else nc.vector)
        proj(wi16, wi16b, 0, xT16, xT16b, xpT16, 0, nc.vector)
        proj(wi16, wi16b, 0, xT16, xT16b, xpT16, 1, nc.vector)
        # mtilde fp8 casts (needed by the first U pair, ~8us after this)
        for c in range(2):
            cs = slice(c * 8, (c + 1) * 8)
            nc.vector.tensor_copy(out=m8a[:, cs, :], in_=m_nat[:, cs, 0:128])
            nc.vector.tensor_copy(out=m8b[:, cs, 0:22], in_=m_nat[:, cs, 128:D])

        # gate weights fp16 (4 g-chunks)
        wg16 = []
        for gi in range(4):
            w = const.tile(
                [wgst[gi].shape[0], G], F16, tag=f"wg16_{gi}", name=f"wg16_{gi}"
            )
            (nc.gpsimd if gi % 2 == 0 else nc.vector).tensor_copy(
                out=w, in_=wgst[gi]
            )
            wg16.append(w)
        bg_half = []
        for gi in range(3):
            t = const.tile(
                [bg_st[gi].shape[0], 1], F32, tag=f"bgh_{gi}", name=f"bgh_{gi}"
            )
            nc.gpsimd.tensor_scalar_mul(out=t, in0=bg_st[gi], scalar1=0.5)
            bg_half.append(t)

        # ---- attention h0 / h1 -------------------------------------------
        # Per half: 16 j-tiles of scores+exp, 8 DoubleRow U pairs.  The exp
        # (ACT) is the rate limiter, so leftover x projections are slotted
        # into h0's PE stream.  U tiles are copied to SBUF right after each
        # half so the single-buffer PSUM pools can be reused.
        U1c, U2c, rr16n = [], [], {}

        def emit_scores(h, j):
            sp = psb.tile([128, HALF], F32, tag="big", name="sp")
            for s in range(2):
                ss = slice(h * HALF + s * 512, h * HALF + (s + 1) * 512)
                nc.tensor.matmul(
                    sp[:, s * 512 : (s + 1) * 512],
                    mpT16[:, j * 128 : (j + 1) * 128], xpT16[:, ss],
                    start=True, stop=True, skip_group_check=True,
                )
            return sp

        def emit_exp(sp, j, e8, slot):
            nc.scalar.activation(
                out=e8[:, :, slot, :], in_=sp, func=Exp,
                bias=maskf[:, j : j + 1], scale=SCALE,
            )

        def emit_U(U1, U2, e8, jp):
            first, last = jp == 0, jp == 7
            for s in range(2):
                ps = slice(s * 512, (s + 1) * 512)
                nc.tensor.matmul(
                    U1[:, ps], m8a[:, 2 * jp : 2 * jp + 2], e8[:, s],
                    start=first, stop=last, perf_mode=DR, skip_group_check=True,
                )
                nc.tensor.matmul(
                    U2[:, ps], m8b[:, 2 * jp : 2 * jp + 2], e8[:, s],
                    start=first, stop=last, perf_mode=DR, skip_group_check=True,
                )

        def emit_denomT(h):
            # denom row [1,1024] -> [128, 8] via PE transposes (so the DVE
            # reciprocal runs partition-parallel instead of 6.5ns/elem on a
            # single lane), then PE-transpose back to a [1, 1024] row.
            # The [1,1] identity sits at partition 32 to match the denom
            # row's base partition (engine APs need 32-aligned bases).
            dps = psb.tile([128, 8], F32, tag="big", name="dps")
            for c in range(8):
                nc.tensor.transpose(
                    dps[:, c : c + 1],
                    U2c[h][32:33, c * 128 : (c + 1) * 128],
                    ident32[32:33, 32:33],
                )
            rrT = work.tile([128, 8], F32, tag="rrT")
            nc.vector.reciprocal(out=rrT, in_=dps)
            rrT16 = work.tile([128, 8], F16, tag="rrT16")
            nc.vector.tensor_copy(out=rrT16, in_=rrT)
            rrps = psb.tile([1, HALF], F16, tag="big", name="rrps")
            for c in range(8):
                nc.tensor.transpose(
                    rrps[:, c * 128 : (c + 1) * 128],
                    rrT16[:, c : c + 1],
                    ident16,
                )
            rr = work.tile([1, HALF], F16, tag="rr16")
            nc.vector.tensor_copy(out=rr[:, 0:512], in_=rrps[:, 0:512])
            nc.scalar.copy(out=rr[:, 512:HALF], in_=rrps[:, 512:HALF])
            rr16n[h] = rr

        # tail stage emitters (interleaved into the next half's attention
        # stream so the PE queue never drains -- a drained PE re-throttles
        # the HAM clock gate to 1.2 GHz)
        o_re = o_d.rearrange("(q n) k -> q n k", n=NB)
        kranges = [(0, 128), (128, 256), (256, G)]
        tails = {}

        def t_bc(h):
            hs = slice(h * HALF, (h + 1) * HALF)
            st = {}
            bc = psb.tile([128, HALF], F32, tag="big", name="bc")
            for s2 in range(2):
                ps = slice(s2 * 512, (s2 + 1) * 512)
                nc.tensor.matmul(
                    bc[:, ps], ones16, rr16n[h][:, ps],
                    start=True, stop=True, skip_group_check=True,
                )
            RC16 = work.tile([128, HALF], F16, tag="RC16")
            nc.vector.tensor_mul(out=RC16, in0=U1c[h], in1=bc)
            RD16 = work.tile([D - 128, HALF], F16, tag="RD16")
            nc.vector.tensor_mul(
                out=RD16, in0=U2c[h][0 : D - 128, :], in1=bc[0 : D - 128, :]
            )
            # output-aligned res staging (partition shifts via DMA)
            res_mid = work.tile([128, HALF], F16, tag="res_mid")
            nc.vector.tensor_copy(out=res_mid[0 : D - 128, :], in_=xT16b[:, hs])
            nc.sync.dma_start(
                out=res_mid[D - 128 : 128, :], in_=RC16[0 : 256 - D, :]
            )
            res_hi = work.tile([G - 256, HALF], F16, tag="res_hi")
            nc.scalar.dma_start(
                out=res_hi[0 : D - 128, :], in_=RC16[256 - D : 128, :]
            )
            nc.sync.dma_start(out=res_hi[D - 128 : G - 256, :], in_=RD16)
            st["res_g"] = [xT16[:, hs], xT16b[:, hs], RC16, RD16]
            st["res_k"] = [xT16[:, hs], res_mid, res_hi]
            st["oT"] = [
                work.tile([128, HALF], F16, tag="oT0", name="oT0"),
                work.tile([128, HALF], F16, tag="oT1", name="oT1"),
                work.tile([G - 256, HALF], F16, tag="oT2", name="oT2"),
            ]
            st["gp"] = {}
            tails[h] = st

        def t_gate_mm(h, kc, s2):
            st = tails[h]
            k0, k1 = kranges[kc]
            if s2 == 0:
                st["gp"][kc] = psb.tile(
                    [k1 - k0, HALF], F32, tag="big", name="gp"
                )
            gp = st["gp"][kc]
            ps = slice(s2 * 512, (s2 + 1) * 512)
            for gc in range(4):
                nc.tensor.matmul(
                    gp[:, ps], wg16[gc][:, k0:k1], st["res_g"][gc][:, ps],
                    start=(gc == 0), stop=(gc == 3), skip_group_check=True,
                )

        def t_gate_act(h, kc):
            # gate = sigmoid(z) = 0.5*(1+tanh(z/2)); tanh shares the exp ACT
            # table.  (gs+1) folds into the DVE multiply, 0.5 into the final
            # PSUM->SBUF copy.
            st = tails[h]
            k0, k1 = kranges[kc]
            gs = work.tile([k1 - k0, HALF], F16, tag="gs", bufs=3)
            nc.scalar.activation(
                out=gs, in_=st["gp"][kc], func=Tanh, bias=bg_half[kc], scale=0.5
            )
            nc.vector.scalar_tensor_tensor(
                out=st["oT"][kc], in0=gs, scalar=1.0, in1=st["res_k"][kc],
                op0=Add, op1=Mult,
            )

        def t_out(h):
            # transpose back (fp16), scale by 0.5 on the upconverting copy,
            # store 4 blocks per DMA
            oT = tails[h]["oT"]
            for g4 in range(2):
                onat = work.tile([128, 4, 304], F32, tag="onat", bufs=2)
                for i in range(2):
                    op2 = psb.tile([128, 2, 304], F16, tag="big", name="op2")
                    for k in range(2):
                        n_loc = g4 * 4 + i * 2 + k
                        sl = slice(n_loc * 128, (n_loc + 1) * 128)
                        nc.tensor.transpose(
                            op2[:, k, 0:128], oT[0][:, sl], ident16
                        )
                        nc.tensor.transpose(
                            op2[:, k, 128:256], oT[1][:, sl], ident16
                        )
                        nc.tensor.transpose(
                            op2[:, k, 256:G], oT[2][:, sl],
                            ident16[: G - 256, : G - 256],
                        )
                    osl = slice(i * 2, (i + 1) * 2)
                    if i == 0:
                        nc.vector.tensor_scalar_mul(
                            out=onat[:, osl, :], in0=op2, scalar1=0.5
                        )
                    else:
                        nc.scalar.activation(
                            out=onat[:, osl, :], in_=op2, func=Copy,
                            bias=0.0, scale=0.5,
                        )
                    n0 = h * 8 + g4 * 4 + i * 2
                    dq = nc.sync if (g4 * 2 + i) % 2 == 0 else nc.scalar
                    dq.dma_start(
                        out=o_re[:, n0 : n0 + 2, :],
                        in_=onat[:, osl, 0:G],
                    )

        for h in range(2):
            U1 = psu.tile([128, HALF], F32, tag="u1")
            U2 = psu.tile([64, HALF], F32, tag="u2")
            sps, e8s = {}, {}
            e8s[0] = epool.tile([128, 2, 2, 512], F8, tag="e8", name="e8")
            sps[0] = emit_scores(h, 0)
            if dbg and h == 0:
                sp0dbg = const.tile([128, HALF], F32)
                nc.vector.tensor_copy(out=sp0dbg, in_=sps[0])
            sps[1] = emit_scores(h, 1)
            for jp in range(8):
                emit_exp(sps.pop(2 * jp), 2 * jp, e8s[jp], 0)
                if 2 * jp + 2 < 16:
                    sps[2 * jp + 2] = emit_scores(h, 2 * jp + 2)
                emit_exp(sps.pop(2 * jp + 1), 2 * jp + 1, e8s[jp], 1)
                if 2 * jp + 3 < 16:
                    sps[2 * jp + 3] = emit_scores(h, 2 * jp + 3)
                if jp + 1 < 8:
                    e8s[jp + 1] = epool.tile([128, 2, 2, 512], F8, tag="e8", name="e8")
                e8cur = e8s.pop(jp)
                if dbg and h == 0 and jp == 0:
                    e8dbg = const.tile([128, 2, 2, 512], F32)
                    nc.vector.tensor_copy(out=e8dbg, in_=e8cur)
                emit_U(U1, U2, e8cur, jp)
                # interleaved fill work: h0 gets the remaining x projections,
                # h1 gets h0's entire gating tail
                if h == 0 and jp in (1, 3):
                    proj(
                        wi16, wi16b, 0, xT16, xT16b, xpT16,
                        2 + (jp - 1) // 2, nc.vector,
                    )
                if h == 1:
                    if jp == 0:
                        emit_denomT(0)
                    elif jp == 1:
                        t_bc(0)
                    elif jp >= 2:
                        kc, s2 = (jp - 2) // 2, jp % 2
                        t_gate_mm(0, kc, s2)
                        if s2 == 1:
                            t_gate_act(0, kc)
            u1c = work.tile([128, HALF], F32, tag="u1c")
            nc.vector.tensor_copy(out=u1c, in_=U1)
            u2c = work.tile([33, HALF], F32, tag="u2c")
            nc.scalar.copy(out=u2c, in_=U2[0:33, :])
            U1c.append(u1c)
            U2c.append(u2c)

        # ---- trailing tail: h0 stores, then all of h1's tail --------------
        emit_denomT(1)
        t_out(0)
        t_bc(1)
        for kc in range(3):
            t_gate_mm(1, kc, 0)
            t_gate_mm(1, kc, 1)
            t_gate_act(1, kc)
        t_out(1)

        if dbg:
            nc.sync.dma_start(out=dbg["u1"][:], in_=U1c[0])
            nc.sync.dma_start(out=dbg["u2"][:], in_=U2c[0])
            rrf = const.tile([1, HALF], F32)
            nc.vector.tensor_copy(out=rrf, in_=rr16n[0])
            nc.sync.dma_start(out=dbg["rr"][:], in_=rrf)
            nc.sync.dma_start(out=dbg["sp0"][:], in_=sp0dbg)
            nc.sync.dma_start(out=dbg["e8"][:], in_=e8dbg)
            m8af = const.tile([128, NB, 128], F32)
            nc.vector.tensor_copy(out=m8af, in_=m8a)
            nc.sync.dma_start(out=dbg["m8a"][:], in_=m8af)
            nc.sync.dma_start(out=dbg["maskf"][:], in_=maskf)


_NC_CACHE = None


def _build_nc():
    global _NC_CACHE
    if _NC_CACHE is not None:
        return _NC_CACHE
    nc = bacc.Bacc(None, target_bir_lowering=False, debug=False)
    x_d = nc.dram_tensor("x", [JX, D], F32, kind="ExternalInput")
    m_d = nc.dram_tensor("m", [JM, D], F32, kind="ExternalInput")
    mask_d = nc.dram_tensor("mask", [JM], I32, kind="ExternalInput")
    wi_d = nc.dram_tensor("Wi", [D, H], F32, kind="ExternalInput")
    bi_d = nc.dram_tensor("bi", [H], F32, kind="ExternalInput")
    wm_d = nc.dram_tensor("Wm", [D, H], F32, kind="ExternalInput")
    bm_d = nc.dram_tensor("bm", [H], F32, kind="ExternalInput")
    wg_d = nc.dram_tensor("Wg", [G, G], F32, kind="ExternalInput")
    bg_d = nc.dram_tensor("bg", [G], F32, kind="ExternalInput")
    o_d = nc.dram_tensor("out", [JX, G], F32, kind="ExternalOutput")
    dbg = None
    if DEBUG_DUMP:
        dbg = {
            "u1": nc.dram_tensor("dbg_u1", [128, HALF], F32, kind="ExternalOutput"),
            "u2": nc.dram_tensor("dbg_u2", [33, HALF], F32, kind="ExternalOutput"),
            "rr": nc.dram_tensor("dbg_rr", [1, HALF], F32, kind="ExternalOutput"),
            "sp0": nc.dram_tensor("dbg_sp0", [128, HALF], F32, kind="ExternalOutput"),
            "e8": nc.dram_tensor("dbg_e8", [128, 2, 2, 512], F32, kind="ExternalOutput"),
            "m8a": nc.dram_tensor("dbg_m8a", [128, NB, 128], F32, kind="ExternalOutput"),
            "maskf": nc.dram_tensor("dbg_maskf", [128, NB], F32, kind="ExternalOutput"),
        }
    with tile.TileContext(nc) as tc:
        _body(tc, x_d, m_d, mask_d, wi_d, bi_d, wm_d, bm_d, wg_d, bg_d, o_d,
              dbg=dbg)
    nc.finalize()
    _NC_CACHE = nc
    return nc


def _in_maps(inputs, memory, mask, Wi, bi, Wm, bm, Wg, bg):
    maps = []
    for b in range(B):
        maps.append(
            {
                "x": np.ascontiguousarray(inputs[b], dtype=np.float32),
                "m": np.ascontiguousarray(memory[b], dtype=np.float32),
                "mask": np.ascontiguousarray(mask[b], dtype=np.int32),
                "Wi": np.ascontiguousarray(Wi, dtype=np.float32),
                "bi": np.ascontiguousarray(bi, dtype=np.float32),
                "Wm": np.ascontiguousarray(Wm, dtype=np.float32),
                "bm": np.ascontiguousarray(bm, dtype=np.float32),
                "Wg": np.ascontiguousarray(Wg, dtype=np.float32),
                "bg": np.ascontiguousarray(bg, dtype=np.float32),
            }
        )
    return maps


def run_spmd(inputs, memory, mask, Wi, bi, Wm, bm, Wg, bg, **spmd_kwargs):
    """Run the kernel across 8 cores; returns the BassKernelResults."""
    nc = _build_nc()
    maps = _in_maps(
        np.asarray(inputs), np.asarray(memory), np.asarray(mask),
        np.asarray(Wi), np.asarray(bi), np.asarray(Wm), np.asarray(bm),
        np.asarray(Wg), np.asarray(bg),
    )
    return run_bass_kernel_spmd(nc, maps, list(range(B)), **spmd_kwargs)


def kernel(inputs, memory, mask, Wi, bi, Wm, bm, Wg, bg):
    res = run_spmd(inputs, memory, mask, Wi, bi, Wm, bm, Wg, bg)
    out = np.stack([res.results[b]["out"] for b in range(B)], axis=0)
    return out.astype(np.float32)
